# revision 1
# baseline (speedup 1.0000x reference)
"""Sparse (sliding-window + sink) GQA attention block on 8 TRN2 NeuronCores.

Sharding: tensor-parallel over the 64 q-heads -> 8 q-heads (= 1 kv-head
group) per core; x replicated; wo partial outputs summed on host.

Per-core dataflow (matmuls in float32r/TF32, storage f32):
  qT[e,s]  = wqT^T x^T   (contraction d on partitions)
  k/vT     = wkvT^T x^T  (k rows 0:64, v rows 64:128 of one [128,1024] tile)
  RoPE on qT/kT (rotate-half swap via SBUF->SBUF DMA, tables from host)
  v natural via PE transpose of vT; ones column appended -> denom for free
  scoresT[j,i] per (h,J): lhsT=kT[:,J], rhs=qT_h[:, J:J+256] (window => 2 blocks)
  eT = exp(scoresT + mask)  (softmax shift-invariance: no max pass)
  attnT/denom = v_aug^T eT into one [65, 2048] psum per head
  window-overlap add, r = 1/(denom+esink) broadcast by ones-matmul, scale,
  out[i,dd] = attnT^T woT -> partial [1024, 2880].  Host: sum partials + wo_b.
"""

import numpy as np

B, S, DIM = 1, 1024, 2880
H, HKV, HD = 64, 8, 64
GROUP = H // HKV
WINDOW = 128
THETA = 150000.0
NC = 8
HL = H // NC                 # 8 local q-heads per core
EL = HL * HD                 # 512 local q-dim
DT = (DIM + 127) // 128      # 23 d-tiles (22 full + 64)
NJ = S // 128                # 8 j/i blocks
MASK_NEG = -1.0e4

_cache = {}


def _build_module():
    import concourse.bacc as bacc
    import concourse.mybir as mybir
    import concourse.tile as tile

    f32 = mybir.dt.float32
    f32r = mybir.dt.float32r
    AF = mybir.ActivationFunctionType
    OP = mybir.AluOpType

    nc = bacc.Bacc("TRN2", target_bir_lowering=False, debug=False)

    def din(name, shape, dt=f32):
        return nc.dram_tensor(name, shape, dt, kind="ExternalInput").ap()

    xT = din("xT", [DIM, S], f32r)           # x^T
    wqT = din("wqT", [128, DT * EL], f32r)   # tiled: [p, 512*t + e]
    wkvT = din("wkvT", [128, DT * 128], f32r)  # tiled: [p, 128*t + (k|v)]
    woT = din("woT", [128, 4 * DIM], f32r)   # tiled: [p, 2880*et + dd]
    qb = din("qb", [128, 4])
    kvb = din("kvb", [128, 1])
    cosq = din("cosq", [128, S])             # 0.125-scaled
    sinq = din("sinq", [128, S])             # 0.125-scaled, sign-baked
    cosk = din("cosk", [64, S])
    sinkt = din("sinkt", [64, S])
    maskT = din("maskT", [128, 256])
    esink = din("esink", [128, 2])           # exp(sinks), row 32*(h%4), col h//4
    id64 = din("id64", [128, 64])            # eye(64) stacked twice
    out_d = nc.dram_tensor("out", [S, DIM], f32, kind="ExternalOutput").ap()

    with tile.TileContext(nc) as tc:
        import contextlib
        with contextlib.ExitStack() as ctx:
            res = ctx.enter_context(tc.tile_pool(name="res", bufs=1))
            wq_sb = res.tile([128, DT * EL], f32r, tag="wq")
            wkv_sb = res.tile([128, DT * 128], f32r, tag="wkv")
            cq_sb = res.tile([128, S], f32, tag="cq")
            sq_sb = res.tile([128, S], f32, tag="sq")
            ck_sb = res.tile([64, S], f32, tag="ck")
            sk_sb = res.tile([64, S], f32, tag="sk")
            mk_sb = res.tile([128, 256], f32, tag="mk")
            qb_sb = res.tile([128, 4], f32, tag="qb")
            kvb_sb = res.tile([128, 1], f32, tag="kvb")
            es_sb = res.tile([128, 2], f32, tag="es")
            id_sb = res.tile([128, 64], f32, tag="id")
            ones0 = res.tile([128, 128], f32, tag="ones0")
            ones_sb = res.tile([128, 128], f32, tag="ones")
            qT = [res.tile([128, S], f32, tag=f"qT{i}", name=f"qT{i}")
                  for i in range(4)]
            kv_sb = res.tile([128, S], f32, tag="kv")
            kv2_sb = res.tile([128, S], f32, tag="kv2")
            v_sb = [res.tile([128, 65], f32, tag=f"v{j}", name=f"v{j}")
                    for j in range(NJ)]
            at_pair = [res.tile([128, S], f32, tag=f"at{t}", name=f"at{t}")
                       for t in range(4)]
            dn_ab = [res.tile([128, S], f32, tag=f"dn{i}", name=f"dn{i}")
                     for i in range(2)]
            rinv_ab = [res.tile([128, S], f32, tag=f"ri{i}", name=f"ri{i}")
                       for i in range(2)]
            wo_sb = res.tile([128, 4 * DIM], f32r, tag="wo")

            nc.sync.dma_start(wq_sb[:], wqT[:])
            nc.sync.dma_start(wkv_sb[:], wkvT[:])
            nc.sync.dma_start(cq_sb[:], cosq[:])
            nc.sync.dma_start(sq_sb[:], sinq[:])
            nc.sync.dma_start(ck_sb[:], cosk[:])
            nc.sync.dma_start(sk_sb[:], sinkt[:])
            nc.sync.dma_start(mk_sb[:], maskT[:])
            nc.sync.dma_start(qb_sb[:], qb[:])
            nc.sync.dma_start(kvb_sb[:], kvb[:])
            nc.sync.dma_start(es_sb[:], esink[:])
            nc.sync.dma_start(id_sb[:], id64[:])
            nc.vector.memset(ones0[:], 1.0)
            nc.vector.tensor_copy(ones_sb[:].bitcast(f32r), ones0[:])
            nc.vector.memset(dn_ab[0][:], 1.0)
            nc.vector.memset(dn_ab[1][:], 1.0)

            # ---------------- Phase A: projections ----------------
            with tc.tile_pool(name="xh", bufs=3) as xh_pool, \
                 tc.tile_pool(name="pqA", bufs=1, space="PSUM") as pq_pool, \
                 tc.tile_pool(name="pkvA", bufs=1, space="PSUM") as pkv_pool:
                for sc in range(2):
                    pq = [pq_pool.tile([128, 512], f32, tag=f"pq{e}",
                                       name=f"pq{e}") for e in range(4)]
                    pkv = pkv_pool.tile([128, 512], f32, tag="pkv")
                    for t in range(DT):
                        dp = 128 if t < DT - 1 else DIM - 128 * (DT - 1)
                        xh = xh_pool.tile([128, 512], f32r, tag="xh")
                        nc.sync.dma_start(
                            xh[:dp, :], xT[128 * t:128 * t + dp,
                                           512 * sc:512 * (sc + 1)])
                        rhs = xh[:dp, :]
                        st, sp = (t == 0), (t == DT - 1)
                        for et in range(4):
                            nc.tensor.matmul(
                                pq[et][:],
                                wq_sb[:dp, EL * t + 128 * et:
                                      EL * t + 128 * (et + 1)],
                                rhs, start=st, stop=sp)
                        nc.tensor.matmul(
                            pkv[:], wkv_sb[:dp, 128 * t:128 * (t + 1)],
                            rhs, start=st, stop=sp)
                    for et in range(4):
                        nc.vector.tensor_scalar_add(
                            qT[et][:, 512 * sc:512 * (sc + 1)].bitcast(f32r),
                            pq[et][:], qb_sb[:, et:et + 1])
                    nc.vector.tensor_scalar_add(
                        kv_sb[:, 512 * sc:512 * (sc + 1)].bitcast(f32r),
                        pkv[:], kvb_sb[:, 0:1])

            # ---------------- Phase B: RoPE (swap via SBUF->SBUF DMA) -----
            with tc.tile_pool(name="rope", bufs=1) as rp:
                for et in range(4):
                    q = qT[et]
                    qsw = rp.tile([128, S], f32, tag="qsw")
                    nc.sync.dma_start(qsw[0:32, :], q[32:64, :])
                    nc.sync.dma_start(qsw[32:64, :], q[0:32, :])
                    nc.sync.dma_start(qsw[64:96, :], q[96:128, :])
                    nc.sync.dma_start(qsw[96:128, :], q[64:96, :])
                    tmp = rp.tile([128, S], f32, tag="tmp")
                    qc = rp.tile([128, S], f32, tag="qc")
                    nc.vector.tensor_tensor(tmp[:], qsw[:], sq_sb[:],
                                            op=OP.mult)
                    nc.vector.tensor_tensor(qc[:], q[:], cq_sb[:], op=OP.mult)
                    nc.vector.tensor_tensor(q[:].bitcast(f32r), qc[:], tmp[:],
                                            op=OP.add)
                # k rope (rows 0:64 of kv_sb)
                ksw = rp.tile([64, S], f32, tag="ksw")
                nc.sync.dma_start(ksw[0:32, :], kv_sb[32:64, :])
                nc.sync.dma_start(ksw[32:64, :], kv_sb[0:32, :])
                tmp = rp.tile([128, S], f32, tag="tmp")
                qc = rp.tile([128, S], f32, tag="qc")
                nc.vector.tensor_tensor(tmp[0:64], ksw[:], sk_sb[:],
                                        op=OP.mult)
                nc.vector.tensor_tensor(qc[0:64], kv_sb[0:64], ck_sb[:],
                                        op=OP.mult)
                nc.vector.tensor_tensor(kv_sb[0:64].bitcast(f32r), qc[0:64],
                                        tmp[0:64], op=OP.add)
                # kT copy at base 64 for odd heads (bit-copy of rounded data)
                nc.sync.dma_start(kv2_sb[64:128, :].bitcast(f32r),
                                  kv_sb[0:64, :].bitcast(f32r))

            # ---------------- Phase B2: v transposes ----------------
            with tc.tile_pool(name="pvt", bufs=2, space="PSUM") as pvt_pool:
                for j in range(NJ):
                    pvt = pvt_pool.tile([128, 64], f32, tag="pvt")
                    nc.tensor.transpose(
                        pvt[:], kv_sb[64:128, 128 * j:128 * (j + 1)],
                        id_sb[64:128, :])
                    nc.vector.tensor_copy(v_sb[j][:, 0:64].bitcast(f32r),
                                          pvt[:])
                    nc.vector.tensor_copy(v_sb[j][:, 64:65].bitcast(f32r),
                                          ones0[:, 0:1])

            # ---------------- Phase C: attention ----------------
            with tc.tile_pool(name="pbig", bufs=1, space="PSUM") as pbig_pool, \
                 tc.tile_pool(name="psc", bufs=2, space="PSUM") as ps_pool, \
                 tc.tile_pool(name="att", bufs=2) as att_pool, \
                 tc.tile_pool(name="set", bufs=3) as set_pool:
                for h in range(HL):
                    qt = qT[h // 2]
                    r0 = 64 * (h % 2)
                    tpi = h // 2
                    kt = kv_sb if h % 2 == 0 else kv2_sb
                    pbig = pbig_pool.tile([65, 2048], f32, tag="pbig")
                    for J in range(NJ):
                        ni = 256 if J < NJ - 1 else 128
                        ps = ps_pool.tile([128, 512], f32, tag="ps")
                        nc.tensor.matmul(
                            ps[:, :ni],
                            kt[r0:r0 + 64, 128 * J:128 * (J + 1)]
                            .bitcast(f32r),
                            qt[r0:r0 + 64, 128 * J:128 * J + ni]
                            .bitcast(f32r),
                            start=True, stop=True)
                        sT = set_pool.tile([128, 256], f32, tag="sT")
                        nc.vector.tensor_tensor(sT[:, :ni], ps[:, :ni],
                                                mk_sb[:, :ni], op=OP.add)
                        eT = set_pool.tile([128, 256], f32, tag="eT")
                        nc.scalar.activation(eT[:, :ni].bitcast(f32r),
                                             sT[:, :ni], AF.Exp)
                        nc.tensor.matmul(
                            pbig[:, 256 * J:256 * J + ni],
                            v_sb[J][:, 0:65].bitcast(f32r),
                            eT[:, :ni].bitcast(f32r),
                            start=True, stop=True)
                    # ---- epilogue: window-overlap adds straight off PSUM
                    dst = at_pair[tpi]
                    dnt = dn_ab[h // 4]
                    dr = 32 * (h % 4)
                    esap = es_sb[dr:dr + 1, (h // 4):(h // 4) + 1]
                    aa_r = att_pool.tile([128, 896], f32, tag="aa")
                    pb_a = pbig[0:64, :].rearrange("p (J x) -> p J x", x=256)
                    pb_d = pbig[64:65, :].rearrange("p (J x) -> p J x", x=256)
                    # right halves of J=0..6 -> SBUF (ACT), then add to left
                    nc.scalar.activation(
                        aa_r[r0:r0 + 64, :].rearrange("p (a b) -> p a b",
                                                      b=128),
                        pb_a[:, 0:7, 128:256], AF.Copy)
                    nc.vector.tensor_copy(
                        dst[r0:r0 + 64, 0:128].bitcast(f32r),
                        pbig[0:64, 0:128])
                    nc.vector.tensor_tensor(
                        dst[r0:r0 + 64, 128:1024].bitcast(f32r)
                        .rearrange("p (a b) -> p a b", b=128),
                        pb_a[:, 1:8, 0:128],
                        aa_r[r0:r0 + 64, :].rearrange("p (a b) -> p a b",
                                                      b=128),
                        op=OP.add)
                    # denom row
                    dnr = att_pool.tile([128, 896], f32, tag="dnr")
                    nc.scalar.activation(
                        dnr[dr:dr + 1, :].rearrange("p (a b) -> p a b",
                                                    b=128),
                        pb_d[:, 0:7, 128:256], AF.Copy)
                    nc.vector.tensor_scalar_add(dnt[dr:dr + 1, 0:128],
                                                pbig[64:65, 0:128], esap)
                    nc.vector.scalar_tensor_tensor(
                        dnt[dr:dr + 1, 128:1024].rearrange(
                            "p (a b) -> p a b", b=128),
                        pb_d[:, 1:8, 0:128], esap,
                        dnr[dr:dr + 1, :].rearrange("p (a b) -> p a b",
                                                    b=128),
                        op0=OP.add, op1=OP.add)

                with nc.allow_low_precision(reason="f32r output for matmul"):
                    nc.vector.reciprocal(rinv_ab[0][:].bitcast(f32r),
                                         dn_ab[0][:])
                    nc.vector.reciprocal(rinv_ab[1][:].bitcast(f32r),
                                         dn_ab[1][:])
                # broadcast r (ones-matmul at base 0) and scale attnT
                with tc.tile_pool(name="rst", bufs=2) as rst_pool:
                    for h in range(HL):
                        t, r0 = h // 2, 64 * (h % 2)
                        dr = 32 * (h % 4)
                        stg = rst_pool.tile([1, S], f32, tag="stg",
                                            name=f"stg{h}")
                        nc.sync.dma_start(
                            stg[:].bitcast(f32r),
                            rinv_ab[h // 4][dr:dr + 1, :].bitcast(f32r))
                        for half in range(2):
                            prt = ps_pool.tile([128, 512], f32, tag="ps")
                            nc.tensor.matmul(
                                prt[0:64, :],
                                ones_sb[0:1, 0:64].bitcast(f32r),
                                stg[0:1, 512 * half:512 * (half + 1)]
                                .bitcast(f32r),
                                start=True, stop=True)
                            nc.vector.tensor_tensor(
                                at_pair[t][r0:r0 + 64,
                                           512 * half:512 * (half + 1)]
                                .bitcast(f32r),
                                at_pair[t][r0:r0 + 64,
                                           512 * half:512 * (half + 1)],
                                prt[0:64, :], op=OP.mult)

            # ---------------- Phase D: output projection ----------------
            nc.sync.dma_start(wo_sb[:], woT[:])
            NDD = 6
            DDC = DIM // NDD  # 480
            with tc.tile_pool(name="po", bufs=3, space="PSUM") as po_pool, \
                 tc.tile_pool(name="ob", bufs=3) as ob_pool:
                for it in range(NJ):
                    for dd in range(NDD):
                        po = po_pool.tile([128, DDC], f32, tag="po")
                        for et in range(4):
                            nc.tensor.matmul(
                                po[:],
                                at_pair[et][:, 128 * it:128 * (it + 1)]
                                .bitcast(f32r),
                                wo_sb[:, DIM * et + DDC * dd:
                                      DIM * et + DDC * (dd + 1)],
                                start=(et == 0), stop=(et == 3))
                        ob = ob_pool.tile([128, DDC], f32, tag="ob")
                        nc.scalar.activation(ob[:], po[:], AF.Copy)
                        nc.sync.dma_start(
                            out_d[128 * it:128 * (it + 1),
                                  DDC * dd:DDC * (dd + 1)], ob[:])

    nc.compile()
    return nc


def _esink_layout(s8):
    out = np.zeros((128, 2), np.float32)
    for h in range(HL):
        out[32 * (h % 4), h // 4] = np.exp(np.float64(s8[h]))
    return out


def _host_prep(x, wq_w, wq_b, wk_w, wk_b, wv_w, wv_b, wo_w, wo_b, sinks):
    """Build per-core input maps (host-side sharding + layout prep)."""
    f = np.float32
    xT = np.ascontiguousarray(x.reshape(S, DIM).T).astype(f)       # [2880,1024]

    half = HD // 2
    inv_freq = 1.0 / (THETA ** (np.arange(half, dtype=np.float64) * 2.0 / HD))
    ang = np.arange(S, dtype=np.float64)[:, None] * inv_freq       # [S, 32]
    cos_t = np.cos(ang).T.astype(f)                                # [32, S]
    sin_t = np.sin(ang).T.astype(f)
    cos64 = np.concatenate([cos_t, cos_t], 0)                      # [64, S]
    sin64 = np.concatenate([-sin_t, sin_t], 0)
    scale = np.float32(HD ** -0.5)
    cosq = np.concatenate([cos64, cos64], 0) * scale               # [128, S]
    sinq = np.concatenate([sin64, sin64], 0) * scale
    cosk = cos64
    sinkt = sin64

    jj = np.arange(128)[:, None]
    ii = np.arange(256)[None, :]
    allow_l = (jj <= ii) & (ii < 128)
    allow_r = (ii >= 128) & (jj > ii - 128)
    maskT = np.where(allow_l | allow_r, 0.0, MASK_NEG).astype(f)

    id64 = np.tile(np.eye(64, dtype=f), (2, 1))

    def tile_T(w):  # [E, DIM] -> tiled transposed [128, DT*E]
        E = w.shape[0]
        out = np.zeros((128, DT * E), f)
        for t in range(DT):
            dp = min(128, DIM - 128 * t)
            out[:dp, E * t:E * (t + 1)] = w[:, 128 * t:128 * t + dp].T
        return out

    in_maps = []
    for c in range(NC):
        wq_c = wq_w[EL * c:EL * (c + 1)]                  # [512, 2880]
        wkv_c = np.concatenate([wk_w[HD * c:HD * (c + 1)],
                                wv_w[HD * c:HD * (c + 1)]], 0)  # [128, 2880]
        wo_c = np.ascontiguousarray(wo_w[:, EL * c:EL * (c + 1)].T)  # [512,2880]
        woT_t = np.zeros((128, 4 * DIM), f)
        for et in range(4):
            woT_t[:, DIM * et:DIM * (et + 1)] = wo_c[128 * et:128 * (et + 1)]
        in_maps.append({
            "xT": xT,
            "wqT": tile_T(wq_c).astype(f),
            "wkvT": tile_T(wkv_c).astype(f),
            "woT": woT_t,
            "qb": np.ascontiguousarray(
                wq_b[EL * c:EL * (c + 1)].reshape(4, 128).T).astype(f),
            "kvb": np.ascontiguousarray(np.concatenate(
                [wk_b[HD * c:HD * (c + 1)],
                 wv_b[HD * c:HD * (c + 1)]]).reshape(1, 128).T).astype(f),
            "cosq": cosq, "sinq": sinq, "cosk": cosk, "sinkt": sinkt,
            "maskT": maskT,
            "esink": _esink_layout(sinks[HL * c:HL * (c + 1)]),
            "id64": id64,
        })
    return in_maps


def run_on_hw(inputs, trace=False, **kw):
    from concourse import bass_utils
    if "nc" not in _cache:
        _cache["nc"] = _build_module()
    in_maps = _host_prep(**inputs)
    res = bass_utils.run_bass_kernel_spmd(
        _cache["nc"], in_maps, core_ids=list(range(NC)), trace=trace, **kw)
    partials = [res.results[c]["out"] for c in range(NC)]
    out = np.sum(np.stack(partials, 0), 0, dtype=np.float64)
    out = (out + inputs["wo_b"].astype(np.float64)).astype(np.float32)
    return out.reshape(B, S, DIM), res


def kernel(**inputs) -> np.ndarray:
    out, _ = run_on_hw(inputs, trace=False)
    return out



# revision 20
# speedup vs baseline: 1.6725x; 1.6725x over previous
"""Sparse (sliding-window + sink) GQA attention block on 8 TRN2 NeuronCores.

Sharding: tensor-parallel over the 64 q-heads -> 8 q-heads (= 1 kv-head
group) per core; x replicated; wo partial outputs summed on host.

All matmuls in bf16 (FWL weight loads, 1 cyc/row at any width); psum f32.
Per-core dataflow:
  A: kv-proj t-loop, then per-et q-proj loops from SBUF-resident packed x
  B: bias-add psum->bf16, RoPE (rotate-half swap via SBUF->SBUF DMA,
     bf16 trig tables with 0.125 q-scale baked), v via PE transpose
  C (pipelined front/back per head): scoresT[j,i] in 4 [128,512] psum
     chunks -> ACT exp psum->bf16 -> 0/1 mask multiply -> split left/right
     pbig matmuls accumulate the window-overlap directly in a [65,1024]
     psum (denom via ones column of v, sink via a spanning rank-1 matmul
     that closes the group) -> reciprocal_approx_fast -> ones-matmul
     broadcast -> scale to at bf16
  D: out[i,dd] partial = sum_et at[et]^T woP, psum->bf16, 8 batched DMAs.
Host: sum bf16 partials (f64) + wo_b.
"""

import numpy as np

B, S, DIM = 1, 1024, 2880
H, HKV, HD = 64, 8, 64
GROUP = H // HKV
WINDOW = 128
THETA = 150000.0
NC = 8
HL = H // NC                 # 8 local q-heads per core
EL = HL * HD                 # 512 local q-dim
DT = (DIM + 127) // 128      # 23 d-tiles (22 full + 64, zero-padded)
NJ = S // 128                # 8 j/i blocks

_cache = {}


def _build_module(taps=False):
    import concourse.bacc as bacc
    import concourse.mybir as mybir
    import concourse.tile as tile

    f32 = mybir.dt.float32
    f32r = mybir.dt.float32r
    bf16 = mybir.dt.bfloat16
    AF = mybir.ActivationFunctionType
    OP = mybir.AluOpType

    nc = bacc.Bacc("TRN2", target_bir_lowering=False, debug=False)

    def din(name, shape, dt=f32):
        return nc.dram_tensor(name, shape, dt, kind="ExternalInput").ap()

    xP = din("xP", [128, DT * 1024], bf16)    # t-major: col 1024t+512sc+c
    wqP = din("wqP", [128, DT * EL], bf16)    # col 512t + e
    wkvP = din("wkvP", [128, DT * 128], bf16)  # col 128t + (k|v)
    woP = din("woP", [128, 4 * DIM], bf16)    # col 2880*et + dd
    qb = din("qb", [128, 4])
    kvb = din("kvb", [128, 1])
    cosq = din("cosq", [128, S], bf16)        # 0.125-scaled
    sinq = din("sinq", [128, S], bf16)        # 0.125-scaled, sign-baked
    cosk = din("cosk", [64, S], bf16)
    sinkt = din("sinkt", [64, S], bf16)
    maskB = din("maskB", [128, 512], bf16)    # 0/1, 256-pattern x2
    es2 = din("es2", [128, HL])               # exp(sinks), replicated rows
    id64 = din("id64", [128, 64], bf16)       # eye(64) stacked twice
    out_d = nc.dram_tensor("out", [S, DIM], bf16, kind="ExternalOutput").ap()
    if taps:
        tap_d = {n: nc.dram_tensor(n, sh, dt, kind="ExternalOutput").ap()
                 for n, sh, dt in [
                     ("d_qbf", [128, 4 * S], bf16), ("d_kb", [128, S], bf16),
                     ("d_vE", [128, NJ * 65], bf16),
                     ("d_eT0", [128, 2048], bf16), ("d_eT1", [128, 2048], bf16),
                     ("d_patv0", [64, S], f32), ("d_patv1", [64, S], f32),
                     ("d_rinv0", [1, S], f32), ("d_rinv1", [1, S], f32),
                     ("d_at", [128, 4 * S], bf16)]}

    xsplit = [0, 3, 7, 11, 15, 19, DT]

    with tile.TileContext(nc) as tc:
        import contextlib
        with contextlib.ExitStack() as ctx:
            res = ctx.enter_context(tc.tile_pool(name="res", bufs=1))
            xp_sb = res.tile([128, DT * 1024], bf16, tag="xp")
            wq_sb = res.tile([128, DT * EL], bf16, tag="wq")
            wkv_sb = res.tile([128, DT * 128], bf16, tag="wkv")
            wo_sb = res.tile([128, 4 * DIM], bf16, tag="wo")
            cq_sb = res.tile([128, S], bf16, tag="cq")
            sq_sb = res.tile([128, S], bf16, tag="sq")
            ck_sb = res.tile([64, S], bf16, tag="ck")
            sk_sb = res.tile([64, S], bf16, tag="sk")
            mk_sb = res.tile([128, 512], bf16, tag="mk")
            qb_sb = res.tile([128, 4], f32, tag="qb")
            kvb_sb = res.tile([128, 1], f32, tag="kvb")
            es_sb = res.tile([128, HL], f32, tag="es")
            id_sb = res.tile([128, 64], bf16, tag="id")
            ones_sb = res.tile([1, S], bf16, tag="ones")
            qbf = [res.tile([128, S], bf16, tag=f"qbf{i}", name=f"qbf{i}")
                   for i in range(4)]
            kb_sb = res.tile([128, S], bf16, tag="kb")
            vE_sb = res.tile([128, NJ * 65], bf16, tag="vE")
            at_sb = [res.tile([128, S], bf16, tag=f"at{i}", name=f"at{i}")
                     for i in range(4)]

            # weights first (kv matmul gates on wkv), then x chunks
            nc.sync.dma_start(wkv_sb[:], wkvP[:])
            for c in range(4):
                w = DT * EL // 4
                nc.sync.dma_start(wq_sb[:, w * c:w * (c + 1)],
                                  wqP[:, w * c:w * (c + 1)])
            for c in range(len(xsplit) - 1):
                c0, c1 = 1024 * xsplit[c], 1024 * xsplit[c + 1]
                nc.sync.dma_start(xp_sb[:, c0:c1], xP[:, c0:c1])
            nc.sync.dma_start(qb_sb[:], qb[:])
            nc.sync.dma_start(kvb_sb[:], kvb[:])
            nc.sync.dma_start(cq_sb[:], cosq[:])
            nc.sync.dma_start(sq_sb[:], sinq[:])
            nc.sync.dma_start(ck_sb[:], cosk[:])
            nc.sync.dma_start(sk_sb[:], sinkt[:])
            nc.sync.dma_start(mk_sb[:], maskB[:])
            nc.sync.dma_start(es_sb[:], es2[:])
            nc.sync.dma_start(id_sb[:], id64[:])
            nc.sync.dma_start(wo_sb[:], woP[:])
            nc.vector.memset(ones_sb[:], 1.0)
            nc.vector.memset(vE_sb[:], 1.0)

            # ---------------- Phase A1: kv projection + rope + v ----------
            with tc.tile_pool(name="pkv", bufs=1, space="PSUM") as pkv_pool, \
                 tc.tile_pool(name="kvt", bufs=1) as kvt_pool, \
                 tc.tile_pool(name="pvt", bufs=2, space="PSUM") as pvt_pool:
                pkv = pkv_pool.tile([128, S], f32, tag="pkv")
                for t in range(DT):
                    for sc in range(2):
                        nc.tensor.matmul(
                            pkv[:, 512 * sc:512 * (sc + 1)],
                            wkv_sb[:, 128 * t:128 * (t + 1)],
                            xp_sb[:, 1024 * t + 512 * sc:
                                  1024 * t + 512 * (sc + 1)],
                            start=(t == 0), stop=(t == DT - 1))
                kv_b = kvt_pool.tile([128, S], bf16, tag="kvb16")
                nc.vector.tensor_scalar_add(kv_b[:], pkv[:], kvb_sb[:, 0:1])
                # k rope (rows 0:64): swap halves via SBUF->SBUF DMA
                ksw = kvt_pool.tile([64, S], bf16, tag="ksw")
                nc.sync.dma_start(ksw[0:32, :], kv_b[32:64, :])
                nc.sync.dma_start(ksw[32:64, :], kv_b[0:32, :])
                kt1 = kvt_pool.tile([64, S], bf16, tag="kt1")
                kt2 = kvt_pool.tile([64, S], bf16, tag="kt2")
                nc.vector.tensor_tensor(kt1[:], ksw[:], sk_sb[:], op=OP.mult)
                nc.vector.tensor_tensor(kt2[:], kv_b[0:64, :], ck_sb[:],
                                        op=OP.mult)
                nc.vector.tensor_tensor(kb_sb[0:64, :], kt1[:], kt2[:],
                                        op=OP.add)
                # replicate kT at partition base 64 for odd heads
                nc.sync.dma_start(kb_sb[64:128, :], kb_sb[0:64, :])
                # v transposes: [64,128] blocks -> [128 j, 64 hd] bf16
                for j in range(NJ):
                    pvt = pvt_pool.tile([128, 64], bf16, tag="pvt")
                    nc.tensor.transpose(
                        pvt[:], kv_b[64:128, 128 * j:128 * (j + 1)],
                        id_sb[64:128, :])
                    nc.vector.tensor_copy(vE_sb[:, 65 * j:65 * j + 64],
                                          pvt[:])

            # ------- Phase A2+B+C: q proj + rope, pipelined attention -----
            with tc.tile_pool(name="pq", bufs=2, space="PSUM") as pq_pool, \
                 tc.tile_pool(name="qt", bufs=2) as qt_pool, \
                 tc.tile_pool(name="psc", bufs=2, space="PSUM") as sc_pool, \
                 tc.tile_pool(name="pat", bufs=2, space="PSUM") as at_pool, \
                 tc.tile_pool(name="eT", bufs=2) as et_pool, \
                 tc.tile_pool(name="er", bufs=2) as er_pool, \
                 tc.tile_pool(name="dnr", bufs=2) as dn_pool:

                def q_proj(et):
                    qt = qt_pool.tile([128, S], bf16, tag="qt")
                    for sc in range(2):
                        pq = pq_pool.tile([128, 512], f32, tag="pq")
                        for t in range(DT):
                            nc.tensor.matmul(
                                pq[:],
                                wq_sb[:, EL * t + 128 * et:
                                      EL * t + 128 * (et + 1)],
                                xp_sb[:, 1024 * t + 512 * sc:
                                      1024 * t + 512 * (sc + 1)],
                                start=(t == 0), stop=(t == DT - 1))
                        nc.vector.tensor_scalar_add(
                            qt[:, 512 * sc:512 * (sc + 1)], pq[:],
                            qb_sb[:, et:et + 1])
                    qsw = qt_pool.tile([128, S], bf16, tag="qsw")
                    nc.sync.dma_start(qsw[0:32, :], qt[32:64, :])
                    nc.sync.dma_start(qsw[32:64, :], qt[0:32, :])
                    nc.sync.dma_start(qsw[64:96, :], qt[96:128, :])
                    nc.sync.dma_start(qsw[96:128, :], qt[64:96, :])
                    t1 = qt_pool.tile([128, S], bf16, tag="t1")
                    t2 = qt_pool.tile([128, S], bf16, tag="t2")
                    nc.vector.tensor_tensor(t1[:], qsw[:], sq_sb[:],
                                            op=OP.mult)
                    nc.vector.tensor_tensor(t2[:], qt[:], cq_sb[:],
                                            op=OP.mult)
                    nc.vector.tensor_tensor(qbf[et][:], t1[:], t2[:],
                                            op=OP.add)

                state = {}

                def head_front(h):
                    qt = qbf[h // 2]
                    r0 = 64 * (h % 2)
                    eT = et_pool.tile([128, 2048], bf16, tag="eT",
                                      name=f"eT{h}")
                    for ch in range(4):
                        ncols = 512 if ch < 3 else 384
                        ps = sc_pool.tile([128, 512], f32, tag="ps")
                        for Jl in range(2):
                            J = 2 * ch + Jl
                            ni = 256 if J < NJ - 1 else 128
                            nc.tensor.matmul(
                                ps[:, 256 * Jl:256 * Jl + ni],
                                kb_sb[r0:r0 + 64, 128 * J:128 * (J + 1)],
                                qt[r0:r0 + 64, 128 * J:128 * J + ni],
                                start=True, stop=True)
                        er = er_pool.tile([128, 512], bf16, tag="er")
                        nc.scalar.activation(
                            er[:, :ncols], ps[:, :ncols], AF.Exp)
                        nc.vector.tensor_tensor(
                            eT[:, 512 * ch:512 * ch + ncols],
                            er[:, :ncols], mk_sb[:, :ncols], op=OP.mult)
                    pat = at_pool.tile([128, S], f32, tag="pat",
                                       name=f"pat{h}")
                    for I in range(NJ):
                        if I > 0:
                            nc.tensor.matmul(
                                pat[0:65, 128 * I:128 * (I + 1)],
                                vE_sb[:, 65 * (I - 1):65 * I],
                                eT[:, 256 * (I - 1) + 128:256 * I],
                                start=True, stop=False)
                        nc.tensor.matmul(
                            pat[0:65, 128 * I:128 * (I + 1)],
                            vE_sb[:, 65 * I:65 * (I + 1)],
                            eT[:, 256 * I:256 * I + 128],
                            start=(I == 0), stop=True)
                    dn = dn_pool.tile([1, S], f32, tag="dn", name=f"dn{h}")
                    nc.vector.tensor_scalar_add(
                        dn[0:1, :], pat[64:65, :], es_sb[64:65, h:h + 1])
                    rinv = dn_pool.tile([1, S], f32, tag="rinv",
                                        name=f"rinv{h}")
                    nc.vector.reciprocal_approx_fast(
                        rinv[0:1, :], dn[0:1, :])
                    rinv_b = dn_pool.tile([1, S], bf16, tag="rinvb",
                                          name=f"rinvb{h}")
                    nc.vector.tensor_copy(rinv_b[:], rinv[:])
                    patv = dn_pool.tile([64, S], f32, tag="patv",
                                        name=f"patv{h}")
                    nc.vector.tensor_copy(patv[:], pat[0:64, :])
                    if taps and h < 2:
                        nc.sync.dma_start(tap_d[f"d_eT{h}"][:], eT[:])
                        nc.sync.dma_start(tap_d[f"d_patv{h}"][:], patv[:])
                        nc.sync.dma_start(tap_d[f"d_rinv{h}"][:], rinv[:])
                    state[h] = (patv, rinv_b, r0)

                def head_back(h):
                    patv, rinv_b, r0 = state.pop(h)
                    for half in range(2):
                        prb = sc_pool.tile([128, 512], f32, tag="ps",
                                           name=f"prb{h}_{half}")
                        nc.tensor.matmul(
                            prb[r0:r0 + 64, :],
                            ones_sb[0:1, 0:64],
                            rinv_b[0:1, 512 * half:512 * (half + 1)],
                            start=True, stop=True)
                        nc.vector.tensor_tensor(
                            at_sb[h // 2][r0:r0 + 64,
                                          512 * half:512 * (half + 1)],
                            patv[0:64, 512 * half:512 * (half + 1)],
                            prb[r0:r0 + 64, :], op=OP.mult)

                q_proj(0)
                q_proj(1)
                head_front(0)
                head_front(1)
                head_back(0)
                q_proj(2)
                head_back(1)
                head_front(2)
                head_front(3)
                head_back(2)
                q_proj(3)
                head_back(3)
                head_front(4)
                head_front(5)
                head_back(4)
                head_front(6)
                head_back(5)
                head_front(7)
                head_back(6)
                head_back(7)

            # ---------------- Phase D: output projection ----------------
            NDD = 6
            DDC = DIM // NDD  # 480
            with tc.tile_pool(name="po", bufs=4, space="PSUM") as po_pool, \
                 tc.tile_pool(name="ob", bufs=2) as ob_pool:
                for it in range(NJ):
                    ob = ob_pool.tile([128, DIM], bf16, tag="ob")
                    for dd in range(NDD):
                        po = po_pool.tile([128, DDC], f32, tag="po")
                        for et in range(4):
                            nc.tensor.matmul(
                                po[:],
                                at_sb[et][:, 128 * it:128 * (it + 1)],
                                wo_sb[:, DIM * et + DDC * dd:
                                      DIM * et + DDC * (dd + 1)],
                                start=(et == 0), stop=(et == 3))
                        if dd % 2 == 0:
                            nc.scalar.activation(
                                ob[:, DDC * dd:DDC * (dd + 1)], po[:],
                                AF.Copy)
                        else:
                            nc.vector.tensor_copy(
                                ob[:, DDC * dd:DDC * (dd + 1)], po[:])
                    nc.sync.dma_start(
                        out_d[128 * it:128 * (it + 1), :], ob[:])
            if taps:
                for i in range(4):
                    nc.sync.dma_start(tap_d["d_qbf"][:, S * i:S * (i + 1)],
                                      qbf[i][:])
                    nc.sync.dma_start(tap_d["d_at"][:, S * i:S * (i + 1)],
                                      at_sb[i][:])
                nc.sync.dma_start(tap_d["d_kb"][:], kb_sb[:])
                nc.sync.dma_start(tap_d["d_vE"][:], vE_sb[:])

    nc.compile()
    return nc


def _host_prep(x, wq_w, wq_b, wk_w, wk_b, wv_w, wv_b, wo_w, wo_b, sinks):
    """Build per-core input maps (host-side sharding + layout prep)."""
    import ml_dtypes
    f = np.float32
    bf = ml_dtypes.bfloat16
    xm = x.reshape(S, DIM).astype(f)

    xP = np.zeros((128, DT * 1024), bf)
    for t in range(DT):
        dp = min(128, DIM - 128 * t)
        blk = xm[:, 128 * t:128 * t + dp].T.astype(bf)     # [dp, S]
        xP[:dp, 1024 * t:1024 * t + 512] = blk[:, 0:512]
        xP[:dp, 1024 * t + 512:1024 * (t + 1)] = blk[:, 512:1024]

    half = HD // 2
    inv_freq = 1.0 / (THETA ** (np.arange(half, dtype=np.float64) * 2.0 / HD))
    ang = np.arange(S, dtype=np.float64)[:, None] * inv_freq
    cos_t = np.cos(ang).T
    sin_t = np.sin(ang).T
    cos64 = np.concatenate([cos_t, cos_t], 0)              # [64, S]
    sin64 = np.concatenate([-sin_t, sin_t], 0)
    scale = HD ** -0.5
    cosq = (np.concatenate([cos64, cos64], 0) * scale).astype(bf)
    sinq = (np.concatenate([sin64, sin64], 0) * scale).astype(bf)
    cosk = cos64.astype(bf)
    sinkt = sin64.astype(bf)

    jj = np.arange(128)[:, None]
    ii = np.arange(256)[None, :]
    allow_l = (jj <= ii) & (ii < 128)
    allow_r = (ii >= 128) & (jj > ii - 128)
    mask256 = (allow_l | allow_r).astype(bf)
    maskB = np.concatenate([mask256, mask256], 1)          # [128, 512]

    id64 = np.tile(np.eye(64), (2, 1)).astype(bf)

    def tile_T(w, E):  # [E, DIM] -> tiled transposed [128, DT*E] bf16
        out = np.zeros((128, DT * E), bf)
        for t in range(DT):
            dp = min(128, DIM - 128 * t)
            out[:dp, E * t:E * (t + 1)] = \
                w[:, 128 * t:128 * t + dp].T.astype(bf)
        return out

    in_maps = []
    for c in range(NC):
        wq_c = wq_w[EL * c:EL * (c + 1)]                  # [512, 2880]
        wkv_c = np.concatenate([wk_w[HD * c:HD * (c + 1)],
                                wv_w[HD * c:HD * (c + 1)]], 0)  # [128, 2880]
        wo_c = np.ascontiguousarray(wo_w[:, EL * c:EL * (c + 1)].T)  # [512,2880]
        woP = np.zeros((128, 4 * DIM), bf)
        for et in range(4):
            woP[:, DIM * et:DIM * (et + 1)] = \
                wo_c[128 * et:128 * (et + 1)].astype(bf)
        es = np.exp(sinks[HL * c:HL * (c + 1)].astype(np.float64)).astype(f)
        es2 = np.tile(es.reshape(1, HL), (128, 1))
        in_maps.append({
            "xP": xP,
            "wqP": tile_T(wq_c, EL),
            "wkvP": tile_T(wkv_c, 128),
            "woP": woP,
            "qb": np.ascontiguousarray(
                wq_b[EL * c:EL * (c + 1)].reshape(4, 128).T).astype(f),
            "kvb": np.ascontiguousarray(np.concatenate(
                [wk_b[HD * c:HD * (c + 1)],
                 wv_b[HD * c:HD * (c + 1)]]).reshape(1, 128).T).astype(f),
            "cosq": cosq, "sinq": sinq, "cosk": cosk, "sinkt": sinkt,
            "maskB": maskB, "es2": es2, "id64": id64,
        })
    return in_maps


def run_on_hw(inputs, trace=False, taps=False, **kw):
    from concourse import bass_utils
    if "nc" not in _cache:
        _cache["nc"] = _build_module(taps=taps)
    in_maps = _host_prep(**inputs)
    res = bass_utils.run_bass_kernel_spmd(
        _cache["nc"], in_maps, core_ids=list(range(NC)), trace=trace, **kw)
    out = np.zeros((S, DIM), np.float64)
    for c in range(NC):
        out += np.asarray(res.results[c]["out"], dtype=np.float64)
    out = (out + inputs["wo_b"].astype(np.float64)).astype(np.float32)
    return out.reshape(B, S, DIM), res


def kernel(**inputs) -> np.ndarray:
    out, _ = run_on_hw(inputs, trace=False)
    return out


# revision 22
# speedup vs baseline: 1.7000x; 1.0164x over previous
"""Sparse (sliding-window + sink) GQA attention block on 8 TRN2 NeuronCores.

Sharding: tensor-parallel over the 64 q-heads -> 8 q-heads (= 1 kv-head
group) per core; x replicated; wo partial outputs summed on host.

All matmuls in bf16 (FWL weight loads, 1 cyc/row at any width); psum f32.
Per-core dataflow:
  A: kv-proj t-loop, then per-et q-proj loops from SBUF-resident packed x
  B: bias-add psum->bf16, RoPE (rotate-half swap via SBUF->SBUF DMA,
     bf16 trig tables with 0.125 q-scale baked), v via PE transpose
  C (pipelined front/back per head): scoresT[j,i] in 4 [128,512] psum
     chunks -> ACT exp psum->bf16 -> 0/1 mask multiply -> split left/right
     pbig matmuls accumulate the window-overlap directly in a [65,1024]
     psum (denom via ones column of v, sink via a spanning rank-1 matmul
     that closes the group) -> reciprocal_approx_fast -> ones-matmul
     broadcast -> scale to at bf16
  D: out[i,dd] partial = sum_et at[et]^T woP, psum->bf16, 8 batched DMAs.
Host: sum bf16 partials (f64) + wo_b.
"""

import numpy as np

B, S, DIM = 1, 1024, 2880
H, HKV, HD = 64, 8, 64
GROUP = H // HKV
WINDOW = 128
THETA = 150000.0
NC = 8
HL = H // NC                 # 8 local q-heads per core
EL = HL * HD                 # 512 local q-dim
DT = (DIM + 127) // 128      # 23 d-tiles (22 full + 64, zero-padded)
NJ = S // 128                # 8 j/i blocks

_cache = {}


def _build_module(taps=False):
    import concourse.bacc as bacc
    import concourse.mybir as mybir
    import concourse.tile as tile

    f32 = mybir.dt.float32
    f32r = mybir.dt.float32r
    bf16 = mybir.dt.bfloat16
    AF = mybir.ActivationFunctionType
    OP = mybir.AluOpType

    nc = bacc.Bacc("TRN2", target_bir_lowering=False, debug=False)

    def din(name, shape, dt=f32):
        return nc.dram_tensor(name, shape, dt, kind="ExternalInput").ap()

    xP = din("xP", [128, DT * 1024], bf16)    # t-major: col 1024t+512sc+c
    wqP = din("wqP", [128, DT * EL], bf16)    # col 512t + e
    wkvP = din("wkvP", [128, DT * 128], bf16)  # col 128t + (k|v)
    woP = din("woP", [128, 4 * DIM], bf16)    # col 2880*et + dd
    qb = din("qb", [128, 4])
    kvb = din("kvb", [128, 1])
    cosq = din("cosq", [128, S], bf16)        # 0.125-scaled
    sinq = din("sinq", [128, S], bf16)        # 0.125-scaled, sign-baked
    cosk = din("cosk", [64, S], bf16)
    sinkt = din("sinkt", [64, S], bf16)
    maskB = din("maskB", [128, 512], bf16)    # 0/1, 256-pattern x2
    esb4 = din("esb4", [128, 2])              # exp(sinks), row 32*(h%4), col h//4
    id64 = din("id64", [128, 64], bf16)       # eye(64) stacked twice
    out_d = nc.dram_tensor("out", [S, DIM], bf16, kind="ExternalOutput").ap()
    if taps:
        tap_d = {n: nc.dram_tensor(n, sh, dt, kind="ExternalOutput").ap()
                 for n, sh, dt in [
                     ("d_qbf", [128, 4 * S], bf16), ("d_kb", [128, S], bf16),
                     ("d_vE", [128, NJ * 65], bf16),
                     ("d_eT0", [128, 2048], bf16), ("d_eT1", [128, 2048], bf16),
                     ("d_patv0", [64, S], bf16), ("d_patv1", [64, S], bf16),
                     ("d_rinv0", [1, S], f32), ("d_rinv1", [1, S], f32),
                     ("d_at", [128, 4 * S], bf16)]}

    xsplit = [0, 3, 7, 11, 15, 19, DT]

    with tile.TileContext(nc) as tc:
        import contextlib
        with contextlib.ExitStack() as ctx:
            res = ctx.enter_context(tc.tile_pool(name="res", bufs=1))
            xp_sb = res.tile([128, DT * 1024], bf16, tag="xp")
            wq_sb = res.tile([128, DT * EL], bf16, tag="wq")
            wkv_sb = res.tile([128, DT * 128], bf16, tag="wkv")
            wo_sb = res.tile([128, 4 * DIM], bf16, tag="wo")
            cq_sb = res.tile([128, S], bf16, tag="cq")
            sq_sb = res.tile([128, S], bf16, tag="sq")
            ck_sb = res.tile([64, S], bf16, tag="ck")
            sk_sb = res.tile([64, S], bf16, tag="sk")
            mk_sb = res.tile([128, 512], bf16, tag="mk")
            qb_sb = res.tile([128, 4], f32, tag="qb")
            kvb_sb = res.tile([128, 1], f32, tag="kvb")
            es_sb = res.tile([128, 2], f32, tag="es")
            id_sb = res.tile([128, 64], bf16, tag="id")
            ones_sb = res.tile([1, S], bf16, tag="ones")
            qbf = [res.tile([128, S], bf16, tag=f"qbf{i}", name=f"qbf{i}")
                   for i in range(4)]
            kb_sb = res.tile([128, S], bf16, tag="kb")
            vE_sb = res.tile([128, NJ * 65], bf16, tag="vE")
            at_sb = [res.tile([128, S], bf16, tag=f"at{i}", name=f"at{i}")
                     for i in range(4)]
            dn_sb = [res.tile([128, S], f32, tag=f"dn{i}", name=f"dn{i}")
                     for i in range(2)]
            rflat = res.tile([1, HL * S], bf16, tag="rflat")

            # wkv + first x chunk gate the kv loop: issue them first,
            # interleave the rest
            nc.sync.dma_start(wkv_sb[:], wkvP[:])
            w = DT * EL // 4
            for c in range(len(xsplit) - 1):
                c0, c1 = 1024 * xsplit[c], 1024 * xsplit[c + 1]
                nc.sync.dma_start(xp_sb[:, c0:c1], xP[:, c0:c1])
                if c < 4:
                    nc.sync.dma_start(wq_sb[:, w * c:w * (c + 1)],
                                      wqP[:, w * c:w * (c + 1)])
            nc.sync.dma_start(qb_sb[:], qb[:])
            nc.sync.dma_start(kvb_sb[:], kvb[:])
            nc.sync.dma_start(cq_sb[:], cosq[:])
            nc.sync.dma_start(sq_sb[:], sinq[:])
            nc.sync.dma_start(ck_sb[:], cosk[:])
            nc.sync.dma_start(sk_sb[:], sinkt[:])
            nc.sync.dma_start(mk_sb[:], maskB[:])
            nc.sync.dma_start(es_sb[:], esb4[:])
            nc.sync.dma_start(id_sb[:], id64[:])
            nc.sync.dma_start(wo_sb[:], woP[:])
            nc.vector.memset(ones_sb[:], 1.0)
            nc.vector.memset(vE_sb[:], 1.0)

            # ---------------- Phase A1: kv projection + rope + v ----------
            with tc.tile_pool(name="pkv", bufs=1, space="PSUM") as pkv_pool, \
                 tc.tile_pool(name="kvt", bufs=1) as kvt_pool, \
                 tc.tile_pool(name="pvt", bufs=2, space="PSUM") as pvt_pool:
                pkv = pkv_pool.tile([128, S], f32, tag="pkv")
                for t in range(DT):
                    for sc in range(2):
                        nc.tensor.matmul(
                            pkv[:, 512 * sc:512 * (sc + 1)],
                            wkv_sb[:, 128 * t:128 * (t + 1)],
                            xp_sb[:, 1024 * t + 512 * sc:
                                  1024 * t + 512 * (sc + 1)],
                            start=(t == 0), stop=(t == DT - 1))
                kv_b = kvt_pool.tile([128, S], bf16, tag="kvb16")
                nc.vector.tensor_scalar_add(kv_b[:], pkv[:], kvb_sb[:, 0:1])
                # k rope (rows 0:64): swap halves via SBUF->SBUF DMA
                ksw = kvt_pool.tile([64, S], bf16, tag="ksw")
                nc.sync.dma_start(ksw[0:32, :], kv_b[32:64, :])
                nc.sync.dma_start(ksw[32:64, :], kv_b[0:32, :])
                kt1 = kvt_pool.tile([64, S], bf16, tag="kt1")
                kt2 = kvt_pool.tile([64, S], bf16, tag="kt2")
                nc.vector.tensor_tensor(kt1[:], ksw[:], sk_sb[:], op=OP.mult)
                nc.vector.tensor_tensor(kt2[:], kv_b[0:64, :], ck_sb[:],
                                        op=OP.mult)
                nc.vector.tensor_tensor(kb_sb[0:64, :], kt1[:], kt2[:],
                                        op=OP.add)
                # replicate kT at partition base 64 for odd heads
                nc.sync.dma_start(kb_sb[64:128, :], kb_sb[0:64, :])
                # v transposes: [64,128] blocks -> [128 j, 64 hd] bf16
                for j in range(NJ):
                    pvt = pvt_pool.tile([128, 64], bf16, tag="pvt")
                    nc.tensor.transpose(
                        pvt[:], kv_b[64:128, 128 * j:128 * (j + 1)],
                        id_sb[64:128, :])
                    nc.vector.tensor_copy(vE_sb[:, 65 * j:65 * j + 64],
                                          pvt[:])

            # ------- Phase A2+B+C: q proj + rope, pipelined attention -----
            with tc.tile_pool(name="pq", bufs=2, space="PSUM") as pq_pool, \
                 tc.tile_pool(name="qt", bufs=2) as qt_pool, \
                 tc.tile_pool(name="psc", bufs=2, space="PSUM") as sc_pool, \
                 tc.tile_pool(name="pat", bufs=2, space="PSUM") as at_pool, \
                 tc.tile_pool(name="eT", bufs=2) as et_pool, \
                 tc.tile_pool(name="er", bufs=2) as er_pool, \
                 tc.tile_pool(name="pv", bufs=8) as pv_pool, \
                 tc.tile_pool(name="nrm", bufs=1) as nm_pool:

                def q_proj(et):
                    qt = qt_pool.tile([128, S], bf16, tag="qt")
                    for sc in range(2):
                        pq = pq_pool.tile([128, 512], f32, tag="pq")
                        for t in range(DT):
                            nc.tensor.matmul(
                                pq[:],
                                wq_sb[:, EL * t + 128 * et:
                                      EL * t + 128 * (et + 1)],
                                xp_sb[:, 1024 * t + 512 * sc:
                                      1024 * t + 512 * (sc + 1)],
                                start=(t == 0), stop=(t == DT - 1))
                        nc.vector.tensor_scalar_add(
                            qt[:, 512 * sc:512 * (sc + 1)], pq[:],
                            qb_sb[:, et:et + 1])
                    qsw = qt_pool.tile([128, S], bf16, tag="qsw")
                    nc.sync.dma_start(qsw[0:32, :], qt[32:64, :])
                    nc.sync.dma_start(qsw[32:64, :], qt[0:32, :])
                    nc.sync.dma_start(qsw[64:96, :], qt[96:128, :])
                    nc.sync.dma_start(qsw[96:128, :], qt[64:96, :])
                    t1 = qt_pool.tile([128, S], bf16, tag="t1")
                    t2 = qt_pool.tile([128, S], bf16, tag="t2")
                    nc.vector.tensor_tensor(t1[:], qsw[:], sq_sb[:],
                                            op=OP.mult)
                    nc.vector.tensor_tensor(t2[:], qt[:], cq_sb[:],
                                            op=OP.mult)
                    nc.vector.tensor_tensor(qbf[et][:], t1[:], t2[:],
                                            op=OP.add)

                state = {}

                def head_front(h):
                    qt = qbf[h // 2]
                    r0 = 64 * (h % 2)
                    eT = et_pool.tile([128, 2048], bf16, tag="eT",
                                      name=f"eT{h}")
                    for ch in range(4):
                        ncols = 512 if ch < 3 else 384
                        ps = sc_pool.tile([128, 512], f32, tag="ps")
                        for Jl in range(2):
                            J = 2 * ch + Jl
                            ni = 256 if J < NJ - 1 else 128
                            nc.tensor.matmul(
                                ps[:, 256 * Jl:256 * Jl + ni],
                                kb_sb[r0:r0 + 64, 128 * J:128 * (J + 1)],
                                qt[r0:r0 + 64, 128 * J:128 * J + ni],
                                start=True, stop=True)
                        er = er_pool.tile([128, 512], bf16, tag="er")
                        nc.scalar.activation(
                            er[:, :ncols], ps[:, :ncols], AF.Exp)
                        nc.gpsimd.tensor_tensor(
                            eT[:, 512 * ch:512 * ch + ncols],
                            er[:, :ncols], mk_sb[:, :ncols], op=OP.mult)
                    pat = at_pool.tile([128, S], f32, tag="pat",
                                       name=f"pat{h}")
                    for I in range(NJ):
                        if I > 0:
                            nc.tensor.matmul(
                                pat[0:65, 128 * I:128 * (I + 1)],
                                vE_sb[:, 65 * (I - 1):65 * I],
                                eT[:, 256 * (I - 1) + 128:256 * I],
                                start=True, stop=False)
                        nc.tensor.matmul(
                            pat[0:65, 128 * I:128 * (I + 1)],
                            vE_sb[:, 65 * I:65 * (I + 1)],
                            eT[:, 256 * I:256 * I + 128],
                            start=(I == 0), stop=True)
                    dr = 32 * (h % 4)
                    nc.vector.tensor_copy(dn_sb[h // 4][dr:dr + 1, :],
                                          pat[64:65, :])
                    patv = pv_pool.tile([64, S], bf16, tag="patv",
                                        name=f"patv{h}")
                    nc.vector.tensor_copy(patv[:], pat[0:64, :])
                    if taps and h < 2:
                        nc.sync.dma_start(tap_d[f"d_eT{h}"][:], eT[:])
                        nc.sync.dma_start(tap_d[f"d_patv{h}"][:], patv[:])
                    state[h] = (patv, r0)

                def norm_batch(b):
                    # 1/(denom+es) for 4 heads at once (rows 0/32/64/96)
                    tmp = nm_pool.tile([128, S], f32, tag="tmpf")
                    rvf = nm_pool.tile([128, S], f32, tag="rvf")
                    rvb = nm_pool.tile([128, S], bf16, tag="rvb")
                    nc.vector.tensor_scalar_add(tmp[:], dn_sb[b][:],
                                                es_sb[:, b:b + 1])
                    nc.vector.reciprocal_approx_fast(rvf[:], tmp[:])
                    nc.vector.tensor_copy(rvb[:], rvf[:])
                    for hl in range(4):
                        h = 4 * b + hl
                        nc.sync.dma_start(
                            rflat[0:1, S * h:S * (h + 1)],
                            rvb[32 * hl:32 * hl + 1, :])
                        if taps and h < 2:
                            nc.sync.dma_start(
                                tap_d[f"d_rinv{h}"][:],
                                rvf[32 * hl:32 * hl + 1, :])

                def head_back(h):
                    patv, r0 = state.pop(h)
                    for half in range(2):
                        prb = sc_pool.tile([128, 512], f32, tag="ps",
                                           name=f"prb{h}_{half}")
                        nc.tensor.matmul(
                            prb[r0:r0 + 64, :],
                            ones_sb[0:1, 0:64],
                            rflat[0:1, S * h + 512 * half:
                                  S * h + 512 * (half + 1)],
                            start=True, stop=True)
                        nc.vector.tensor_tensor(
                            at_sb[h // 2][r0:r0 + 64,
                                          512 * half:512 * (half + 1)],
                            patv[0:64, 512 * half:512 * (half + 1)],
                            prb[r0:r0 + 64, :], op=OP.mult)

                q_proj(0)
                q_proj(1)
                head_front(0)
                head_front(1)
                q_proj(2)
                head_front(2)
                head_front(3)
                norm_batch(0)
                q_proj(3)
                head_back(0)
                head_front(4)
                head_back(1)
                head_front(5)
                head_back(2)
                head_front(6)
                head_back(3)
                head_front(7)
                norm_batch(1)
                head_back(4)
                head_back(5)
                head_back(6)
                head_back(7)

            # ---------------- Phase D: output projection ----------------
            NDD = 6
            DDC = DIM // NDD  # 480
            with tc.tile_pool(name="po", bufs=4, space="PSUM") as po_pool, \
                 tc.tile_pool(name="ob", bufs=2) as ob_pool:
                for it in range(NJ):
                    ob = ob_pool.tile([128, DIM], bf16, tag="ob")
                    for dd in range(NDD):
                        po = po_pool.tile([128, DDC], f32, tag="po")
                        for et in range(4):
                            nc.tensor.matmul(
                                po[:],
                                at_sb[et][:, 128 * it:128 * (it + 1)],
                                wo_sb[:, DIM * et + DDC * dd:
                                      DIM * et + DDC * (dd + 1)],
                                start=(et == 0), stop=(et == 3))
                        if dd % 2 == 0:
                            nc.scalar.activation(
                                ob[:, DDC * dd:DDC * (dd + 1)], po[:],
                                AF.Copy)
                        else:
                            nc.vector.tensor_copy(
                                ob[:, DDC * dd:DDC * (dd + 1)], po[:])
                    nc.sync.dma_start(
                        out_d[128 * it:128 * (it + 1), :], ob[:])
            if taps:
                for i in range(4):
                    nc.sync.dma_start(tap_d["d_qbf"][:, S * i:S * (i + 1)],
                                      qbf[i][:])
                    nc.sync.dma_start(tap_d["d_at"][:, S * i:S * (i + 1)],
                                      at_sb[i][:])
                nc.sync.dma_start(tap_d["d_kb"][:], kb_sb[:])
                nc.sync.dma_start(tap_d["d_vE"][:], vE_sb[:])

    nc.compile()
    return nc


def _host_prep(x, wq_w, wq_b, wk_w, wk_b, wv_w, wv_b, wo_w, wo_b, sinks):
    """Build per-core input maps (host-side sharding + layout prep)."""
    import ml_dtypes
    f = np.float32
    bf = ml_dtypes.bfloat16
    xm = x.reshape(S, DIM).astype(f)

    xP = np.zeros((128, DT * 1024), bf)
    for t in range(DT):
        dp = min(128, DIM - 128 * t)
        blk = xm[:, 128 * t:128 * t + dp].T.astype(bf)     # [dp, S]
        xP[:dp, 1024 * t:1024 * t + 512] = blk[:, 0:512]
        xP[:dp, 1024 * t + 512:1024 * (t + 1)] = blk[:, 512:1024]

    half = HD // 2
    inv_freq = 1.0 / (THETA ** (np.arange(half, dtype=np.float64) * 2.0 / HD))
    ang = np.arange(S, dtype=np.float64)[:, None] * inv_freq
    cos_t = np.cos(ang).T
    sin_t = np.sin(ang).T
    cos64 = np.concatenate([cos_t, cos_t], 0)              # [64, S]
    sin64 = np.concatenate([-sin_t, sin_t], 0)
    scale = HD ** -0.5
    cosq = (np.concatenate([cos64, cos64], 0) * scale).astype(bf)
    sinq = (np.concatenate([sin64, sin64], 0) * scale).astype(bf)
    cosk = cos64.astype(bf)
    sinkt = sin64.astype(bf)

    jj = np.arange(128)[:, None]
    ii = np.arange(256)[None, :]
    allow_l = (jj <= ii) & (ii < 128)
    allow_r = (ii >= 128) & (jj > ii - 128)
    mask256 = (allow_l | allow_r).astype(bf)
    maskB = np.concatenate([mask256, mask256], 1)          # [128, 512]

    id64 = np.tile(np.eye(64), (2, 1)).astype(bf)

    def tile_T(w, E):  # [E, DIM] -> tiled transposed [128, DT*E] bf16
        out = np.zeros((128, DT * E), bf)
        for t in range(DT):
            dp = min(128, DIM - 128 * t)
            out[:dp, E * t:E * (t + 1)] = \
                w[:, 128 * t:128 * t + dp].T.astype(bf)
        return out

    in_maps = []
    for c in range(NC):
        wq_c = wq_w[EL * c:EL * (c + 1)]                  # [512, 2880]
        wkv_c = np.concatenate([wk_w[HD * c:HD * (c + 1)],
                                wv_w[HD * c:HD * (c + 1)]], 0)  # [128, 2880]
        wo_c = np.ascontiguousarray(wo_w[:, EL * c:EL * (c + 1)].T)  # [512,2880]
        woP = np.zeros((128, 4 * DIM), bf)
        for et in range(4):
            woP[:, DIM * et:DIM * (et + 1)] = \
                wo_c[128 * et:128 * (et + 1)].astype(bf)
        es = np.exp(sinks[HL * c:HL * (c + 1)].astype(np.float64)).astype(f)
        esb4 = np.zeros((128, 2), f)
        for h in range(HL):
            esb4[32 * (h % 4), h // 4] = es[h]
        in_maps.append({
            "xP": xP,
            "wqP": tile_T(wq_c, EL),
            "wkvP": tile_T(wkv_c, 128),
            "woP": woP,
            "qb": np.ascontiguousarray(
                wq_b[EL * c:EL * (c + 1)].reshape(4, 128).T).astype(f),
            "kvb": np.ascontiguousarray(np.concatenate(
                [wk_b[HD * c:HD * (c + 1)],
                 wv_b[HD * c:HD * (c + 1)]]).reshape(1, 128).T).astype(f),
            "cosq": cosq, "sinq": sinq, "cosk": cosk, "sinkt": sinkt,
            "maskB": maskB, "esb4": esb4, "id64": id64,
        })
    return in_maps


def run_on_hw(inputs, trace=False, taps=False, **kw):
    from concourse import bass_utils
    if "nc" not in _cache:
        _cache["nc"] = _build_module(taps=taps)
    in_maps = _host_prep(**inputs)
    res = bass_utils.run_bass_kernel_spmd(
        _cache["nc"], in_maps, core_ids=list(range(NC)), trace=trace, **kw)
    out = np.zeros((S, DIM), np.float64)
    for c in range(NC):
        out += np.asarray(res.results[c]["out"], dtype=np.float64)
    out = (out + inputs["wo_b"].astype(np.float64)).astype(np.float32)
    return out.reshape(B, S, DIM), res


def kernel(**inputs) -> np.ndarray:
    out, _ = run_on_hw(inputs, trace=False)
    return out


# revision 23
# speedup vs baseline: 1.7731x; 1.0430x over previous
"""Sparse (sliding-window + sink) GQA attention block on 8 TRN2 NeuronCores.

Sharding: tensor-parallel over the 64 q-heads -> 8 q-heads (= 1 kv-head
group) per core; x replicated; wo partial outputs summed on host.

All matmuls in bf16 (FWL weight loads, 1 cyc/row at any width); psum f32.
Per-core dataflow:
  A: kv-proj t-loop, then per-et q-proj loops from SBUF-resident packed x
  B: bias-add psum->bf16, RoPE (rotate-half swap via SBUF->SBUF DMA,
     bf16 trig tables with 0.125 q-scale baked), v via PE transpose
  C (pipelined front/back per head): scoresT[j,i] in 4 [128,512] psum
     chunks -> ACT exp psum->bf16 -> 0/1 mask multiply -> split left/right
     pbig matmuls accumulate the window-overlap directly in a [65,1024]
     psum (denom via ones column of v, sink via a spanning rank-1 matmul
     that closes the group) -> reciprocal_approx_fast -> ones-matmul
     broadcast -> scale to at bf16
  D: out[i,dd] partial = sum_et at[et]^T woP, psum->bf16, 8 batched DMAs.
Host: sum bf16 partials (f64) + wo_b.
"""

import numpy as np

B, S, DIM = 1, 1024, 2880
H, HKV, HD = 64, 8, 64
GROUP = H // HKV
WINDOW = 128
THETA = 150000.0
NC = 8
HL = H // NC                 # 8 local q-heads per core
EL = HL * HD                 # 512 local q-dim
DT = (DIM + 127) // 128      # 23 d-tiles (22 full + 64, zero-padded)
NJ = S // 128                # 8 j/i blocks

_cache = {}


def _build_module(taps=False):
    import concourse.bacc as bacc
    import concourse.mybir as mybir
    import concourse.tile as tile

    f32 = mybir.dt.float32
    f32r = mybir.dt.float32r
    bf16 = mybir.dt.bfloat16
    AF = mybir.ActivationFunctionType
    OP = mybir.AluOpType

    nc = bacc.Bacc("TRN2", target_bir_lowering=False, debug=False)

    def din(name, shape, dt=f32):
        return nc.dram_tensor(name, shape, dt, kind="ExternalInput").ap()

    xP = din("xP", [128, DT * 1024], bf16)    # t-major: col 1024t+512sc+c
    wqP = din("wqP", [128, DT * EL], bf16)    # col 512t + e
    wkvP = din("wkvP", [128, DT * 128], bf16)  # col 128t + (k|v)
    woP = din("woP", [128, 4 * DIM], bf16)    # col 2880*et + dd
    qb = din("qb", [128, 4])
    kvb = din("kvb", [128, 1])
    cosq = din("cosq", [128, S], bf16)        # 0.125-scaled
    sinq = din("sinq", [128, S], bf16)        # 0.125-scaled, sign-baked
    cosk = din("cosk", [64, S], bf16)
    sinkt = din("sinkt", [64, S], bf16)
    maskB = din("maskB", [128, 512], bf16)    # 0/1, 256-pattern x2
    esb4 = din("esb4", [128, 2])              # exp(sinks), row 32*(h%4), col h//4
    id64 = din("id64", [128, 64], bf16)       # eye(64) stacked twice
    out_d = nc.dram_tensor("out", [S, DIM], bf16, kind="ExternalOutput").ap()
    if taps:
        tap_d = {n: nc.dram_tensor(n, sh, dt, kind="ExternalOutput").ap()
                 for n, sh, dt in [
                     ("d_qbf", [128, 4 * S], bf16), ("d_kb", [128, S], bf16),
                     ("d_vE", [128, NJ * 65], bf16),
                     ("d_eT0", [128, 2048], bf16), ("d_eT1", [128, 2048], bf16),
                     ("d_patv0", [65, S], bf16), ("d_patv1", [65, S], bf16),
                     ("d_rinv0", [1, S], f32), ("d_rinv1", [1, S], f32),
                     ("d_at", [128, 4 * S], bf16)]}

    xsplit = [0, 3, 7, 11, 15, 19, DT]

    with tile.TileContext(nc) as tc:
        import contextlib
        with contextlib.ExitStack() as ctx:
            res = ctx.enter_context(tc.tile_pool(name="res", bufs=1))
            xp_sb = res.tile([128, DT * 1024], bf16, tag="xp")
            wq_sb = res.tile([128, DT * EL], bf16, tag="wq")
            wkv_sb = res.tile([128, DT * 128], bf16, tag="wkv")
            wo_sb = res.tile([128, 4 * DIM], bf16, tag="wo")
            cq_sb = res.tile([128, S], bf16, tag="cq")
            sq_sb = res.tile([128, S], bf16, tag="sq")
            ck_sb = res.tile([64, S], bf16, tag="ck")
            sk_sb = res.tile([64, S], bf16, tag="sk")
            mk_sb = res.tile([128, 512], bf16, tag="mk")
            qb_sb = res.tile([128, 4], f32, tag="qb")
            kvb_sb = res.tile([128, 1], f32, tag="kvb")
            es_sb = res.tile([128, 2], f32, tag="es")
            id_sb = res.tile([128, 64], bf16, tag="id")
            ones_sb = res.tile([1, S], bf16, tag="ones")
            qbf = [res.tile([128, S], bf16, tag=f"qbf{i}", name=f"qbf{i}")
                   for i in range(4)]
            kb_sb = res.tile([128, S], bf16, tag="kb")
            vE_sb = res.tile([128, NJ * 65], bf16, tag="vE")
            at_sb = [res.tile([128, S], bf16, tag=f"at{i}", name=f"at{i}")
                     for i in range(4)]
            dn_sb = [res.tile([128, S], bf16, tag=f"dn{i}", name=f"dn{i}")
                     for i in range(2)]
            rflat = res.tile([1, HL * S], bf16, tag="rflat")

            # wkv + x gate the kv loop: issue them first; wq next;
            # trig/smalls after; wo is deferred until attention starts
            nc.sync.dma_start(wkv_sb[:], wkvP[:])
            for c in range(len(xsplit) - 1):
                c0, c1 = 1024 * xsplit[c], 1024 * xsplit[c + 1]
                nc.sync.dma_start(xp_sb[:, c0:c1], xP[:, c0:c1])
            w = DT * EL // 4
            for c in range(4):
                nc.sync.dma_start(wq_sb[:, w * c:w * (c + 1)],
                                  wqP[:, w * c:w * (c + 1)])
            nc.sync.dma_start(kvb_sb[:], kvb[:])
            nc.sync.dma_start(ck_sb[:], cosk[:])
            nc.sync.dma_start(sk_sb[:], sinkt[:])
            nc.sync.dma_start(id_sb[:], id64[:])
            nc.sync.dma_start(qb_sb[:], qb[:])
            nc.sync.dma_start(cq_sb[:], cosq[:])
            nc.sync.dma_start(sq_sb[:], sinq[:])
            nc.sync.dma_start(mk_sb[:], maskB[:])
            nc.sync.dma_start(es_sb[:], esb4[:])
            nc.vector.memset(ones_sb[:], 1.0)
            nc.vector.memset(vE_sb[:], 1.0)

            # ---------------- Phase A1: kv projection + rope + v ----------
            with tc.tile_pool(name="pkv", bufs=1, space="PSUM") as pkv_pool, \
                 tc.tile_pool(name="kvt", bufs=1) as kvt_pool, \
                 tc.tile_pool(name="pvt", bufs=2, space="PSUM") as pvt_pool:
                pkv = pkv_pool.tile([128, S], f32, tag="pkv")
                for t in range(DT):
                    for sc in range(2):
                        nc.tensor.matmul(
                            pkv[:, 512 * sc:512 * (sc + 1)],
                            wkv_sb[:, 128 * t:128 * (t + 1)],
                            xp_sb[:, 1024 * t + 512 * sc:
                                  1024 * t + 512 * (sc + 1)],
                            start=(t == 0), stop=(t == DT - 1))
                kv_b = kvt_pool.tile([128, S], bf16, tag="kvb16")
                nc.vector.tensor_scalar_add(kv_b[:], pkv[:], kvb_sb[:, 0:1])
                # k rope (rows 0:64): swap halves via SBUF->SBUF DMA
                ksw = kvt_pool.tile([64, S], bf16, tag="ksw")
                nc.scalar.dma_start(ksw[0:32, :], kv_b[32:64, :])
                nc.scalar.dma_start(ksw[32:64, :], kv_b[0:32, :])
                kt1 = kvt_pool.tile([64, S], bf16, tag="kt1")
                kt2 = kvt_pool.tile([64, S], bf16, tag="kt2")
                nc.vector.tensor_tensor(kt1[:], ksw[:], sk_sb[:], op=OP.mult)
                nc.vector.tensor_tensor(kt2[:], kv_b[0:64, :], ck_sb[:],
                                        op=OP.mult)
                nc.vector.tensor_tensor(kb_sb[0:64, :], kt1[:], kt2[:],
                                        op=OP.add)
                # replicate kT at partition base 64 for odd heads
                nc.scalar.dma_start(kb_sb[64:128, :], kb_sb[0:64, :])
                # v transposes: [64,128] blocks -> [128 j, 64 hd] bf16
                for j in range(NJ):
                    pvt = pvt_pool.tile([128, 64], bf16, tag="pvt")
                    nc.tensor.transpose(
                        pvt[:], kv_b[64:128, 128 * j:128 * (j + 1)],
                        id_sb[64:128, :])
                    nc.vector.tensor_copy(vE_sb[:, 65 * j:65 * j + 64],
                                          pvt[:])

            # ------- Phase A2+B+C: q proj + rope, pipelined attention -----
            with tc.tile_pool(name="pq", bufs=2, space="PSUM") as pq_pool, \
                 tc.tile_pool(name="qt", bufs=2) as qt_pool, \
                 tc.tile_pool(name="psc", bufs=2, space="PSUM") as sc_pool, \
                 tc.tile_pool(name="pat", bufs=2, space="PSUM") as at_pool, \
                 tc.tile_pool(name="eT", bufs=2) as et_pool, \
                 tc.tile_pool(name="er", bufs=2) as er_pool, \
                 tc.tile_pool(name="pv", bufs=8) as pv_pool, \
                 tc.tile_pool(name="nrm", bufs=1) as nm_pool:

                def q_proj(et):
                    qt = qt_pool.tile([128, S], bf16, tag="qt")
                    for sc in range(2):
                        pq = pq_pool.tile([128, 512], f32, tag="pq")
                        for t in range(DT):
                            nc.tensor.matmul(
                                pq[:],
                                wq_sb[:, EL * t + 128 * et:
                                      EL * t + 128 * (et + 1)],
                                xp_sb[:, 1024 * t + 512 * sc:
                                      1024 * t + 512 * (sc + 1)],
                                start=(t == 0), stop=(t == DT - 1))
                        nc.vector.tensor_scalar_add(
                            qt[:, 512 * sc:512 * (sc + 1)], pq[:],
                            qb_sb[:, et:et + 1])
                    qsw = qt_pool.tile([128, S], bf16, tag="qsw")
                    nc.scalar.dma_start(qsw[0:32, :], qt[32:64, :])
                    nc.scalar.dma_start(qsw[32:64, :], qt[0:32, :])
                    nc.scalar.dma_start(qsw[64:96, :], qt[96:128, :])
                    nc.scalar.dma_start(qsw[96:128, :], qt[64:96, :])
                    t1 = qt_pool.tile([128, S], bf16, tag="t1")
                    t2 = qt_pool.tile([128, S], bf16, tag="t2")
                    nc.vector.tensor_tensor(t1[:], qsw[:], sq_sb[:],
                                            op=OP.mult)
                    nc.vector.tensor_tensor(t2[:], qt[:], cq_sb[:],
                                            op=OP.mult)
                    nc.vector.tensor_tensor(qbf[et][:], t1[:], t2[:],
                                            op=OP.add)

                state = {}
                wo_loaded = []

                def head_front(h):
                    if not wo_loaded:
                        nc.sync.dma_start(wo_sb[:], woP[:])
                        wo_loaded.append(True)
                    qt = qbf[h // 2]
                    r0 = 64 * (h % 2)
                    eT = et_pool.tile([128, 2048], bf16, tag="eT",
                                      name=f"eT{h}")
                    for ch in range(4):
                        ncols = 512 if ch < 3 else 384
                        ps = sc_pool.tile([128, 512], f32, tag="ps")
                        for Jl in range(2):
                            J = 2 * ch + Jl
                            ni = 256 if J < NJ - 1 else 128
                            nc.tensor.matmul(
                                ps[:, 256 * Jl:256 * Jl + ni],
                                kb_sb[r0:r0 + 64, 128 * J:128 * (J + 1)],
                                qt[r0:r0 + 64, 128 * J:128 * J + ni],
                                start=True, stop=True)
                        er = er_pool.tile([128, 512], bf16, tag="er")
                        nc.scalar.activation(
                            er[:, :ncols], ps[:, :ncols], AF.Exp)
                        eng = nc.gpsimd if ch == 3 else nc.vector
                        eng.tensor_tensor(
                            eT[:, 512 * ch:512 * ch + ncols],
                            er[:, :ncols], mk_sb[:, :ncols], op=OP.mult)
                    pat = at_pool.tile([128, S], f32, tag="pat",
                                       name=f"pat{h}")
                    for I in range(NJ):
                        if I > 0:
                            nc.tensor.matmul(
                                pat[0:65, 128 * I:128 * (I + 1)],
                                vE_sb[:, 65 * (I - 1):65 * I],
                                eT[:, 256 * (I - 1) + 128:256 * I],
                                start=True, stop=False)
                        nc.tensor.matmul(
                            pat[0:65, 128 * I:128 * (I + 1)],
                            vE_sb[:, 65 * I:65 * (I + 1)],
                            eT[:, 256 * I:256 * I + 128],
                            start=(I == 0), stop=True)
                    dr = 32 * (h % 4)
                    patv = pv_pool.tile([65, S], bf16, tag="patv",
                                        name=f"patv{h}")
                    nc.vector.tensor_copy(patv[:], pat[0:65, :])
                    nc.scalar.dma_start(dn_sb[h // 4][dr:dr + 1, :],
                                        patv[64:65, :])
                    if taps and h < 2:
                        nc.sync.dma_start(tap_d[f"d_eT{h}"][:], eT[:])
                        nc.sync.dma_start(tap_d[f"d_patv{h}"][:], patv[:])
                    state[h] = (patv, r0)

                def norm_batch(b):
                    # 1/(denom+es) for 4 heads at once (rows 0/32/64/96)
                    tmp = nm_pool.tile([128, S], f32, tag="tmpf")
                    rvf = nm_pool.tile([128, S], f32, tag="rvf")
                    rvb = nm_pool.tile([128, S], bf16, tag="rvb")
                    nc.vector.tensor_scalar_add(tmp[:], dn_sb[b][:],
                                                es_sb[:, b:b + 1])
                    nc.vector.reciprocal_approx_fast(rvf[:], tmp[:])
                    nc.vector.tensor_copy(rvb[:], rvf[:])
                    for hl in range(4):
                        h = 4 * b + hl
                        nc.scalar.dma_start(
                            rflat[0:1, S * h:S * (h + 1)],
                            rvb[32 * hl:32 * hl + 1, :])
                        if taps and h < 2:
                            nc.sync.dma_start(
                                tap_d[f"d_rinv{h}"][:],
                                rvf[32 * hl:32 * hl + 1, :])

                def head_back(h):
                    patv, r0 = state.pop(h)
                    for half in range(2):
                        prb = sc_pool.tile([128, 512], f32, tag="ps",
                                           name=f"prb{h}_{half}")
                        nc.tensor.matmul(
                            prb[r0:r0 + 64, :],
                            ones_sb[0:1, 0:64],
                            rflat[0:1, S * h + 512 * half:
                                  S * h + 512 * (half + 1)],
                            start=True, stop=True)
                        nc.vector.tensor_tensor(
                            at_sb[h // 2][r0:r0 + 64,
                                          512 * half:512 * (half + 1)],
                            patv[0:64, 512 * half:512 * (half + 1)],
                            prb[r0:r0 + 64, :], op=OP.mult)

                q_proj(0)
                q_proj(1)
                head_front(0)
                head_front(1)
                q_proj(2)
                head_front(2)
                head_front(3)
                norm_batch(0)
                q_proj(3)
                head_back(0)
                head_front(4)
                head_back(1)
                head_front(5)
                head_back(2)
                head_front(6)
                head_back(3)
                head_front(7)
                norm_batch(1)
                head_back(4)
                head_back(5)
                head_back(6)
                head_back(7)

            # ---------------- Phase D: output projection ----------------
            NDD = 6
            DDC = DIM // NDD  # 480
            with tc.tile_pool(name="po", bufs=4, space="PSUM") as po_pool, \
                 tc.tile_pool(name="ob", bufs=2) as ob_pool:
                for it in range(NJ):
                    ob = ob_pool.tile([128, DIM], bf16, tag="ob")
                    for dd in range(NDD):
                        po = po_pool.tile([128, DDC], f32, tag="po")
                        for et in range(4):
                            nc.tensor.matmul(
                                po[:],
                                at_sb[et][:, 128 * it:128 * (it + 1)],
                                wo_sb[:, DIM * et + DDC * dd:
                                      DIM * et + DDC * (dd + 1)],
                                start=(et == 0), stop=(et == 3))
                        if dd % 2 == 0:
                            nc.scalar.activation(
                                ob[:, DDC * dd:DDC * (dd + 1)], po[:],
                                AF.Copy)
                        else:
                            nc.vector.tensor_copy(
                                ob[:, DDC * dd:DDC * (dd + 1)], po[:])
                    nc.sync.dma_start(
                        out_d[128 * it:128 * (it + 1), :], ob[:])
            if taps:
                for i in range(4):
                    nc.sync.dma_start(tap_d["d_qbf"][:, S * i:S * (i + 1)],
                                      qbf[i][:])
                    nc.sync.dma_start(tap_d["d_at"][:, S * i:S * (i + 1)],
                                      at_sb[i][:])
                nc.sync.dma_start(tap_d["d_kb"][:], kb_sb[:])
                nc.sync.dma_start(tap_d["d_vE"][:], vE_sb[:])

    nc.compile()
    return nc


def _host_prep(x, wq_w, wq_b, wk_w, wk_b, wv_w, wv_b, wo_w, wo_b, sinks):
    """Build per-core input maps (host-side sharding + layout prep)."""
    import ml_dtypes
    f = np.float32
    bf = ml_dtypes.bfloat16
    xm = x.reshape(S, DIM).astype(f)

    xP = np.zeros((128, DT * 1024), bf)
    for t in range(DT):
        dp = min(128, DIM - 128 * t)
        blk = xm[:, 128 * t:128 * t + dp].T.astype(bf)     # [dp, S]
        xP[:dp, 1024 * t:1024 * t + 512] = blk[:, 0:512]
        xP[:dp, 1024 * t + 512:1024 * (t + 1)] = blk[:, 512:1024]

    half = HD // 2
    inv_freq = 1.0 / (THETA ** (np.arange(half, dtype=np.float64) * 2.0 / HD))
    ang = np.arange(S, dtype=np.float64)[:, None] * inv_freq
    cos_t = np.cos(ang).T
    sin_t = np.sin(ang).T
    cos64 = np.concatenate([cos_t, cos_t], 0)              # [64, S]
    sin64 = np.concatenate([-sin_t, sin_t], 0)
    scale = HD ** -0.5
    cosq = (np.concatenate([cos64, cos64], 0) * scale).astype(bf)
    sinq = (np.concatenate([sin64, sin64], 0) * scale).astype(bf)
    cosk = cos64.astype(bf)
    sinkt = sin64.astype(bf)

    jj = np.arange(128)[:, None]
    ii = np.arange(256)[None, :]
    allow_l = (jj <= ii) & (ii < 128)
    allow_r = (ii >= 128) & (jj > ii - 128)
    mask256 = (allow_l | allow_r).astype(bf)
    maskB = np.concatenate([mask256, mask256], 1)          # [128, 512]

    id64 = np.tile(np.eye(64), (2, 1)).astype(bf)

    def tile_T(w, E):  # [E, DIM] -> tiled transposed [128, DT*E] bf16
        out = np.zeros((128, DT * E), bf)
        for t in range(DT):
            dp = min(128, DIM - 128 * t)
            out[:dp, E * t:E * (t + 1)] = \
                w[:, 128 * t:128 * t + dp].T.astype(bf)
        return out

    in_maps = []
    for c in range(NC):
        wq_c = wq_w[EL * c:EL * (c + 1)]                  # [512, 2880]
        wkv_c = np.concatenate([wk_w[HD * c:HD * (c + 1)],
                                wv_w[HD * c:HD * (c + 1)]], 0)  # [128, 2880]
        wo_c = np.ascontiguousarray(wo_w[:, EL * c:EL * (c + 1)].T)  # [512,2880]
        woP = np.zeros((128, 4 * DIM), bf)
        for et in range(4):
            woP[:, DIM * et:DIM * (et + 1)] = \
                wo_c[128 * et:128 * (et + 1)].astype(bf)
        es = np.exp(sinks[HL * c:HL * (c + 1)].astype(np.float64)).astype(f)
        esb4 = np.zeros((128, 2), f)
        for h in range(HL):
            esb4[32 * (h % 4), h // 4] = es[h]
        in_maps.append({
            "xP": xP,
            "wqP": tile_T(wq_c, EL),
            "wkvP": tile_T(wkv_c, 128),
            "woP": woP,
            "qb": np.ascontiguousarray(
                wq_b[EL * c:EL * (c + 1)].reshape(4, 128).T).astype(f),
            "kvb": np.ascontiguousarray(np.concatenate(
                [wk_b[HD * c:HD * (c + 1)],
                 wv_b[HD * c:HD * (c + 1)]]).reshape(1, 128).T).astype(f),
            "cosq": cosq, "sinq": sinq, "cosk": cosk, "sinkt": sinkt,
            "maskB": maskB, "esb4": esb4, "id64": id64,
        })
    return in_maps


def run_on_hw(inputs, trace=False, taps=False, **kw):
    from concourse import bass_utils
    if "nc" not in _cache:
        _cache["nc"] = _build_module(taps=taps)
    in_maps = _host_prep(**inputs)
    res = bass_utils.run_bass_kernel_spmd(
        _cache["nc"], in_maps, core_ids=list(range(NC)), trace=trace, **kw)
    out = np.zeros((S, DIM), np.float64)
    for c in range(NC):
        out += np.asarray(res.results[c]["out"], dtype=np.float64)
    out = (out + inputs["wo_b"].astype(np.float64)).astype(np.float32)
    return out.reshape(B, S, DIM), res


def kernel(**inputs) -> np.ndarray:
    out, _ = run_on_hw(inputs, trace=False)
    return out


# revision 25
# speedup vs baseline: 1.8071x; 1.0192x over previous
"""Sparse (sliding-window + sink) GQA attention block on 8 TRN2 NeuronCores.

Sharding: tensor-parallel over the 64 q-heads -> 8 q-heads (= 1 kv-head
group) per core; x replicated; wo partial outputs summed on host.

All matmuls in bf16 (FWL weight loads, 1 cyc/row at any width); psum f32.
Per-core dataflow:
  A: kv-proj t-loop, then per-et q-proj loops from SBUF-resident packed x
  B: bias-add psum->bf16, RoPE (rotate-half swap via SBUF->SBUF DMA,
     bf16 trig tables with 0.125 q-scale baked), v via PE transpose
  C (pipelined front/back per head): scoresT[j,i] in 4 [128,512] psum
     chunks -> ACT exp psum->bf16 -> 0/1 mask multiply -> split left/right
     pbig matmuls accumulate the window-overlap directly in a [65,1024]
     psum (denom via ones column of v, sink via a spanning rank-1 matmul
     that closes the group) -> reciprocal_approx_fast -> ones-matmul
     broadcast -> scale to at bf16
  D: out[i,dd] partial = sum_et at[et]^T woP, psum->bf16, 8 batched DMAs.
Host: sum bf16 partials (f64) + wo_b.
"""

import numpy as np

B, S, DIM = 1, 1024, 2880
H, HKV, HD = 64, 8, 64
GROUP = H // HKV
WINDOW = 128
THETA = 150000.0
NC = 8
HL = H // NC                 # 8 local q-heads per core
EL = HL * HD                 # 512 local q-dim
DT = (DIM + 127) // 128      # 23 d-tiles (22 full + 64, zero-padded)
NJ = S // 128                # 8 j/i blocks

_cache = {}


def _build_module(taps=False):
    import concourse.bacc as bacc
    import concourse.mybir as mybir
    import concourse.tile as tile

    f32 = mybir.dt.float32
    f32r = mybir.dt.float32r
    bf16 = mybir.dt.bfloat16
    AF = mybir.ActivationFunctionType
    OP = mybir.AluOpType

    nc = bacc.Bacc("TRN2", target_bir_lowering=False, debug=False)

    def din(name, shape, dt=f32):
        return nc.dram_tensor(name, shape, dt, kind="ExternalInput").ap()

    xP = din("xP", [128, DT * 1024], bf16)    # t-major: col 1024t+512sc+c
    wqP = din("wqP", [128, DT * EL], bf16)    # col 512t + e
    wkvP = din("wkvP", [128, DT * 128], bf16)  # col 128t + (k|v)
    woP = din("woP", [128, 4 * DIM], bf16)    # col 2880*et + dd
    qb = din("qb", [128, 4])
    kvb = din("kvb", [128, 1])
    cosq = din("cosq", [128, S], bf16)        # 0.125-scaled
    sinq = din("sinq", [128, S], bf16)        # 0.125-scaled, sign-baked
    cosk = din("cosk", [64, S], bf16)
    sinkt = din("sinkt", [64, S], bf16)
    maskB = din("maskB", [128, 512], bf16)    # 0/1, 256-pattern x2
    esb4 = din("esb4", [128, 4])              # exp(sinks) layouts
    id64 = din("id64", [128, 64], bf16)       # eye(64) stacked twice
    out_d = nc.dram_tensor("out", [S, DIM], bf16, kind="ExternalOutput").ap()
    if taps:
        tap_d = {n: nc.dram_tensor(n, sh, dt, kind="ExternalOutput").ap()
                 for n, sh, dt in [
                     ("d_qbf", [128, 4 * S], bf16), ("d_kb", [128, S], bf16),
                     ("d_vE", [128, NJ * 65], bf16),
                     ("d_eT0", [128, 2048], bf16), ("d_eT1", [128, 2048], bf16),
                     ("d_patv0", [65, S], bf16), ("d_patv1", [65, S], bf16),
                     ("d_rinv0", [1, S], f32), ("d_rinv1", [1, S], f32),
                     ("d_at", [128, 4 * S], bf16)]}

    xsplit = [0, 2, 5, 8, 11, 14, 17, 20, DT]

    with tile.TileContext(nc) as tc:
        import contextlib
        with contextlib.ExitStack() as ctx:
            res = ctx.enter_context(tc.tile_pool(name="res", bufs=1))
            xp_sb = res.tile([128, DT * 1024], bf16, tag="xp")
            wq_sb = res.tile([128, DT * EL], bf16, tag="wq")
            wkv_sb = res.tile([128, DT * 128], bf16, tag="wkv")
            wo_sb = res.tile([128, 4 * DIM], bf16, tag="wo")
            cq_sb = res.tile([128, S], bf16, tag="cq")
            sq_sb = res.tile([128, S], bf16, tag="sq")
            ck_sb = res.tile([64, S], bf16, tag="ck")
            sk_sb = res.tile([64, S], bf16, tag="sk")
            mk_sb = res.tile([128, 512], bf16, tag="mk")
            qb_sb = res.tile([128, 4], f32, tag="qb")
            kvb_sb = res.tile([128, 1], f32, tag="kvb")
            es_sb = res.tile([128, 4], f32, tag="es")
            id_sb = res.tile([128, 64], bf16, tag="id")
            ones_sb = res.tile([1, S], bf16, tag="ones")
            qbf = [res.tile([128, S], bf16, tag=f"qbf{i}", name=f"qbf{i}")
                   for i in range(4)]
            kb_sb = res.tile([128, S], bf16, tag="kb")
            vE_sb = res.tile([128, NJ * 65], bf16, tag="vE")
            at_sb = [res.tile([128, S], bf16, tag=f"at{i}", name=f"at{i}")
                     for i in range(4)]
            dn_sb = [res.tile([128, S], bf16, tag=f"dn{i}", name=f"dn{i}")
                     for i in range(2)]
            rflat = res.tile([1, HL * S], bf16, tag="rflat")

            # wkv + x gate the kv loop: issue them first; wq next;
            # trig/smalls after; wo is deferred until attention starts
            nc.sync.dma_start(wkv_sb[:, :128 * 8], wkvP[:, :128 * 8])
            nc.sync.dma_start(wkv_sb[:, 128 * 8:], wkvP[:, 128 * 8:])
            for c in range(len(xsplit) - 1):
                c0, c1 = 1024 * xsplit[c], 1024 * xsplit[c + 1]
                nc.sync.dma_start(xp_sb[:, c0:c1], xP[:, c0:c1])
            nc.sync.dma_start(kvb_sb[:], kvb[:])
            nc.sync.dma_start(ck_sb[:], cosk[:])
            nc.sync.dma_start(sk_sb[:], sinkt[:])
            nc.sync.dma_start(id_sb[:], id64[:])
            nc.sync.dma_start(qb_sb[:], qb[:])
            w = DT * EL // 4
            for c in range(4):
                nc.sync.dma_start(wq_sb[:, w * c:w * (c + 1)],
                                  wqP[:, w * c:w * (c + 1)])
            nc.sync.dma_start(cq_sb[:], cosq[:])
            nc.sync.dma_start(sq_sb[:], sinq[:])
            nc.sync.dma_start(mk_sb[:], maskB[:])
            nc.sync.dma_start(es_sb[:], esb4[:])
            nc.vector.memset(ones_sb[:], 1.0)
            nc.vector.memset(vE_sb[:], 1.0)

            # ---------------- Phase A1: kv projection + rope + v ----------
            with tc.tile_pool(name="pkv", bufs=1, space="PSUM") as pkv_pool, \
                 tc.tile_pool(name="kvt", bufs=1) as kvt_pool, \
                 tc.tile_pool(name="pvt", bufs=2, space="PSUM") as pvt_pool:
                pkv = pkv_pool.tile([128, S], f32, tag="pkv")
                for t in range(DT):
                    for sc in range(2):
                        nc.tensor.matmul(
                            pkv[:, 512 * sc:512 * (sc + 1)],
                            wkv_sb[:, 128 * t:128 * (t + 1)],
                            xp_sb[:, 1024 * t + 512 * sc:
                                  1024 * t + 512 * (sc + 1)],
                            start=(t == 0), stop=(t == DT - 1))
                kv_b = kvt_pool.tile([128, S], bf16, tag="kvb16")
                nc.vector.tensor_scalar_add(kv_b[:], pkv[:], kvb_sb[:, 0:1])
                # k rope (rows 0:64): swap halves via SBUF->SBUF DMA
                ksw = kvt_pool.tile([64, S], bf16, tag="ksw")
                nc.scalar.dma_start(ksw[0:32, :], kv_b[32:64, :])
                nc.scalar.dma_start(ksw[32:64, :], kv_b[0:32, :])
                kt1 = kvt_pool.tile([64, S], bf16, tag="kt1")
                kt2 = kvt_pool.tile([64, S], bf16, tag="kt2")
                nc.vector.tensor_tensor(kt1[:], ksw[:], sk_sb[:], op=OP.mult)
                nc.vector.tensor_tensor(kt2[:], kv_b[0:64, :], ck_sb[:],
                                        op=OP.mult)
                nc.vector.tensor_tensor(kb_sb[0:64, :], kt1[:], kt2[:],
                                        op=OP.add)
                # replicate kT at partition base 64 for odd heads
                nc.scalar.dma_start(kb_sb[64:128, :], kb_sb[0:64, :])
                # v transposes: [64,128] blocks -> [128 j, 64 hd] bf16
                for j in range(NJ):
                    pvt = pvt_pool.tile([128, 64], bf16, tag="pvt")
                    nc.tensor.transpose(
                        pvt[:], kv_b[64:128, 128 * j:128 * (j + 1)],
                        id_sb[64:128, :])
                    nc.vector.tensor_copy(vE_sb[:, 65 * j:65 * j + 64],
                                          pvt[:])

            # ------- Phase A2+B+C: q proj + rope, pipelined attention -----
            with tc.tile_pool(name="pq", bufs=2, space="PSUM") as pq_pool, \
                 tc.tile_pool(name="qt", bufs=2) as qt_pool, \
                 tc.tile_pool(name="psc", bufs=2, space="PSUM") as sc_pool, \
                 tc.tile_pool(name="pat", bufs=2, space="PSUM") as at_pool, \
                 tc.tile_pool(name="eT", bufs=2) as et_pool, \
                 tc.tile_pool(name="er", bufs=2) as er_pool, \
                 tc.tile_pool(name="pv", bufs=8) as pv_pool, \
                 tc.tile_pool(name="nrm", bufs=1) as nm_pool, \
                 tc.tile_pool(name="rbd", bufs=2) as rb_pool:

                def q_proj(et):
                    qt = qt_pool.tile([128, S], bf16, tag="qt")
                    for sc in range(2):
                        pq = pq_pool.tile([128, 512], f32, tag="pq")
                        for t in range(DT):
                            nc.tensor.matmul(
                                pq[:],
                                wq_sb[:, EL * t + 128 * et:
                                      EL * t + 128 * (et + 1)],
                                xp_sb[:, 1024 * t + 512 * sc:
                                      1024 * t + 512 * (sc + 1)],
                                start=(t == 0), stop=(t == DT - 1))
                        nc.vector.tensor_scalar_add(
                            qt[:, 512 * sc:512 * (sc + 1)], pq[:],
                            qb_sb[:, et:et + 1])
                    qsw = qt_pool.tile([128, S], bf16, tag="qsw")
                    nc.scalar.dma_start(qsw[0:32, :], qt[32:64, :])
                    nc.scalar.dma_start(qsw[32:64, :], qt[0:32, :])
                    nc.scalar.dma_start(qsw[64:96, :], qt[96:128, :])
                    nc.scalar.dma_start(qsw[96:128, :], qt[64:96, :])
                    t1 = qt_pool.tile([128, S], bf16, tag="t1")
                    t2 = qt_pool.tile([128, S], bf16, tag="t2")
                    nc.vector.tensor_tensor(t1[:], qsw[:], sq_sb[:],
                                            op=OP.mult)
                    nc.vector.tensor_tensor(t2[:], qt[:], cq_sb[:],
                                            op=OP.mult)
                    nc.vector.tensor_tensor(qbf[et][:], t1[:], t2[:],
                                            op=OP.add)

                state = {}
                wo_loaded = []

                def head_front(h):
                    if not wo_loaded:
                        nc.sync.dma_start(wo_sb[:], woP[:])
                        wo_loaded.append(True)
                    qt = qbf[h // 2]
                    r0 = 64 * (h % 2)
                    eT = et_pool.tile([128, 2048], bf16, tag="eT",
                                      name=f"eT{h}")
                    for ch in range(4):
                        ncols = 512 if ch < 3 else 384
                        ps = sc_pool.tile([128, 512], f32, tag="ps")
                        for Jl in range(2):
                            J = 2 * ch + Jl
                            ni = 256 if J < NJ - 1 else 128
                            nc.tensor.matmul(
                                ps[:, 256 * Jl:256 * Jl + ni],
                                kb_sb[r0:r0 + 64, 128 * J:128 * (J + 1)],
                                qt[r0:r0 + 64, 128 * J:128 * J + ni],
                                start=True, stop=True)
                        er = er_pool.tile([128, 512], bf16, tag="er")
                        nc.scalar.activation(
                            er[:, :ncols], ps[:, :ncols], AF.Exp)
                        eng = nc.gpsimd if ch == 3 else nc.vector
                        eng.tensor_tensor(
                            eT[:, 512 * ch:512 * ch + ncols],
                            er[:, :ncols], mk_sb[:, :ncols], op=OP.mult)
                    pat = at_pool.tile([128, S], f32, tag="pat",
                                       name=f"pat{h}")
                    for I in range(NJ):
                        if I > 0:
                            nc.tensor.matmul(
                                pat[0:65, 128 * I:128 * (I + 1)],
                                vE_sb[:, 65 * (I - 1):65 * I],
                                eT[:, 256 * (I - 1) + 128:256 * I],
                                start=True, stop=False)
                        nc.tensor.matmul(
                            pat[0:65, 128 * I:128 * (I + 1)],
                            vE_sb[:, 65 * I:65 * (I + 1)],
                            eT[:, 256 * I:256 * I + 128],
                            start=(I == 0), stop=True)
                    patv = pv_pool.tile([65, S], bf16, tag="patv",
                                        name=f"patv{h}")
                    nc.vector.tensor_copy(patv[:], pat[0:65, :])
                    rbd = None
                    if h < 6:
                        dr = 32 * (h % 4)
                        nc.scalar.dma_start(dn_sb[h // 4][dr:dr + 1, :],
                                            patv[64:65, :])
                    else:
                        rbd = norm_direct(h, pat)
                    if taps and h < 2:
                        nc.sync.dma_start(tap_d[f"d_eT{h}"][:], eT[:])
                        nc.sync.dma_start(tap_d[f"d_patv{h}"][:], patv[:])
                    state[h] = (patv, r0, rbd)

                def norm_batch(b, nh):
                    # 1/(denom+es) for nh heads at once (rows 32*hl)
                    tmp = nm_pool.tile([128, S], f32, tag="tmpf")
                    rvf = nm_pool.tile([128, S], f32, tag="rvf")
                    rvb = nm_pool.tile([128, S], bf16, tag="rvb")
                    nc.vector.tensor_scalar_add(tmp[:], dn_sb[b][:],
                                                es_sb[:, b:b + 1])
                    nc.vector.reciprocal_approx_fast(rvf[:], tmp[:])
                    nc.vector.tensor_copy(rvb[:], rvf[:])
                    for hl in range(nh):
                        h = 4 * b + hl
                        nc.scalar.dma_start(
                            rflat[0:1, S * h:S * (h + 1)],
                            rvb[32 * hl:32 * hl + 1, :])
                        if taps and h < 2:
                            nc.sync.dma_start(
                                tap_d[f"d_rinv{h}"][:],
                                rvf[32 * hl:32 * hl + 1, :])

                def norm_direct(h, pat):
                    # short DVE-only chain for the last heads
                    dnd = nm_pool.tile([128, S], f32, tag="tmpf",
                                       name=f"dnd{h}")
                    nc.vector.tensor_scalar_add(
                        dnd[0:1, :], pat[64:65, :],
                        es_sb[64:65, h - 4:h - 3])
                    rvd = nm_pool.tile([128, S], f32, tag="rvf",
                                       name=f"rvd{h}")
                    nc.vector.reciprocal_approx_fast(rvd[0:1, :],
                                                     dnd[0:1, :])
                    rbd = rb_pool.tile([1, S], bf16, tag="rbd",
                                       name=f"rbd{h}")
                    nc.vector.tensor_copy(rbd[:], rvd[0:1, :])
                    return rbd

                def head_back(h):
                    patv, r0, rbd = state.pop(h)
                    for half in range(2):
                        prb = sc_pool.tile([128, 512], f32, tag="ps",
                                           name=f"prb{h}_{half}")
                        rsrc = (rflat[0:1, S * h + 512 * half:
                                      S * h + 512 * (half + 1)]
                                if rbd is None else
                                rbd[0:1, 512 * half:512 * (half + 1)])
                        nc.tensor.matmul(
                            prb[r0:r0 + 64, :],
                            ones_sb[0:1, 0:64],
                            rsrc,
                            start=True, stop=True)
                        nc.vector.tensor_tensor(
                            at_sb[h // 2][r0:r0 + 64,
                                          512 * half:512 * (half + 1)],
                            patv[0:64, 512 * half:512 * (half + 1)],
                            prb[r0:r0 + 64, :], op=OP.mult)

                q_proj(0)
                q_proj(1)
                head_front(0)
                head_front(1)
                q_proj(2)
                head_front(2)
                head_front(3)
                norm_batch(0, 4)
                q_proj(3)
                head_front(4)
                head_front(5)
                norm_batch(1, 2)
                head_back(0)
                head_back(1)
                head_front(6)
                head_back(2)
                head_front(7)
                head_back(3)
                head_back(4)
                head_back(5)
                head_back(6)
                head_back(7)

            # ---------------- Phase D: output projection ----------------
            NDD = 6
            DDC = DIM // NDD  # 480
            with tc.tile_pool(name="po", bufs=4, space="PSUM") as po_pool, \
                 tc.tile_pool(name="ob", bufs=2) as ob_pool:
                for it in range(NJ):
                    ob = ob_pool.tile([128, DIM], bf16, tag="ob")
                    for dd in range(NDD):
                        po = po_pool.tile([128, DDC], f32, tag="po")
                        for et in range(4):
                            nc.tensor.matmul(
                                po[:],
                                at_sb[et][:, 128 * it:128 * (it + 1)],
                                wo_sb[:, DIM * et + DDC * dd:
                                      DIM * et + DDC * (dd + 1)],
                                start=(et == 0), stop=(et == 3))
                        if dd % 2 == 0:
                            nc.scalar.activation(
                                ob[:, DDC * dd:DDC * (dd + 1)], po[:],
                                AF.Copy)
                        else:
                            nc.vector.tensor_copy(
                                ob[:, DDC * dd:DDC * (dd + 1)], po[:])
                    nc.sync.dma_start(
                        out_d[128 * it:128 * (it + 1), :], ob[:])
            if taps:
                for i in range(4):
                    nc.sync.dma_start(tap_d["d_qbf"][:, S * i:S * (i + 1)],
                                      qbf[i][:])
                    nc.sync.dma_start(tap_d["d_at"][:, S * i:S * (i + 1)],
                                      at_sb[i][:])
                nc.sync.dma_start(tap_d["d_kb"][:], kb_sb[:])
                nc.sync.dma_start(tap_d["d_vE"][:], vE_sb[:])

    nc.compile()
    return nc


def _host_prep(x, wq_w, wq_b, wk_w, wk_b, wv_w, wv_b, wo_w, wo_b, sinks):
    """Build per-core input maps (host-side sharding + layout prep)."""
    import ml_dtypes
    f = np.float32
    bf = ml_dtypes.bfloat16
    xm = x.reshape(S, DIM).astype(f)

    xP = np.zeros((128, DT * 1024), bf)
    for t in range(DT):
        dp = min(128, DIM - 128 * t)
        blk = xm[:, 128 * t:128 * t + dp].T.astype(bf)     # [dp, S]
        xP[:dp, 1024 * t:1024 * t + 512] = blk[:, 0:512]
        xP[:dp, 1024 * t + 512:1024 * (t + 1)] = blk[:, 512:1024]

    half = HD // 2
    inv_freq = 1.0 / (THETA ** (np.arange(half, dtype=np.float64) * 2.0 / HD))
    ang = np.arange(S, dtype=np.float64)[:, None] * inv_freq
    cos_t = np.cos(ang).T
    sin_t = np.sin(ang).T
    cos64 = np.concatenate([cos_t, cos_t], 0)              # [64, S]
    sin64 = np.concatenate([-sin_t, sin_t], 0)
    scale = HD ** -0.5
    cosq = (np.concatenate([cos64, cos64], 0) * scale).astype(bf)
    sinq = (np.concatenate([sin64, sin64], 0) * scale).astype(bf)
    cosk = cos64.astype(bf)
    sinkt = sin64.astype(bf)

    jj = np.arange(128)[:, None]
    ii = np.arange(256)[None, :]
    allow_l = (jj <= ii) & (ii < 128)
    allow_r = (ii >= 128) & (jj > ii - 128)
    mask256 = (allow_l | allow_r).astype(bf)
    maskB = np.concatenate([mask256, mask256], 1)          # [128, 512]

    id64 = np.tile(np.eye(64), (2, 1)).astype(bf)

    def tile_T(w, E):  # [E, DIM] -> tiled transposed [128, DT*E] bf16
        out = np.zeros((128, DT * E), bf)
        for t in range(DT):
            dp = min(128, DIM - 128 * t)
            out[:dp, E * t:E * (t + 1)] = \
                w[:, 128 * t:128 * t + dp].T.astype(bf)
        return out

    in_maps = []
    for c in range(NC):
        wq_c = wq_w[EL * c:EL * (c + 1)]                  # [512, 2880]
        wkv_c = np.concatenate([wk_w[HD * c:HD * (c + 1)],
                                wv_w[HD * c:HD * (c + 1)]], 0)  # [128, 2880]
        wo_c = np.ascontiguousarray(wo_w[:, EL * c:EL * (c + 1)].T)  # [512,2880]
        woP = np.zeros((128, 4 * DIM), bf)
        for et in range(4):
            woP[:, DIM * et:DIM * (et + 1)] = \
                wo_c[128 * et:128 * (et + 1)].astype(bf)
        es = np.exp(sinks[HL * c:HL * (c + 1)].astype(np.float64)).astype(f)
        esb4 = np.zeros((128, 4), f)
        for h in range(6):
            esb4[32 * (h % 4), h // 4] = es[h]
        esb4[64, 2] = es[6]
        esb4[64, 3] = es[7]
        in_maps.append({
            "xP": xP,
            "wqP": tile_T(wq_c, EL),
            "wkvP": tile_T(wkv_c, 128),
            "woP": woP,
            "qb": np.ascontiguousarray(
                wq_b[EL * c:EL * (c + 1)].reshape(4, 128).T).astype(f),
            "kvb": np.ascontiguousarray(np.concatenate(
                [wk_b[HD * c:HD * (c + 1)],
                 wv_b[HD * c:HD * (c + 1)]]).reshape(1, 128).T).astype(f),
            "cosq": cosq, "sinq": sinq, "cosk": cosk, "sinkt": sinkt,
            "maskB": maskB, "esb4": esb4, "id64": id64,
        })
    return in_maps


def run_on_hw(inputs, trace=False, taps=False, **kw):
    from concourse import bass_utils
    if "nc" not in _cache:
        _cache["nc"] = _build_module(taps=taps)
    in_maps = _host_prep(**inputs)
    res = bass_utils.run_bass_kernel_spmd(
        _cache["nc"], in_maps, core_ids=list(range(NC)), trace=trace, **kw)
    out = np.zeros((S, DIM), np.float64)
    for c in range(NC):
        out += np.asarray(res.results[c]["out"], dtype=np.float64)
    out = (out + inputs["wo_b"].astype(np.float64)).astype(np.float32)
    return out.reshape(B, S, DIM), res


def kernel(**inputs) -> np.ndarray:
    out, _ = run_on_hw(inputs, trace=False)
    return out
